# revision 4
# baseline (speedup 1.0000x reference)
"""Trainium2 Bass kernel for GroupNorm + single-head self-attention block.

Reference computation (per batch b):
    xn = GroupNorm(x; 32 groups over (L, C/32)) * gn_scale + gn_bias
    q, k, v = xn@wq+bq, xn@wk+bk, xn@wv+bv
    out = softmax(q k^T / sqrt(C)) v @ wo + bo + x

Sharding: 8 cores = 4 batches x 2 query-halves. Each core receives its
batch's [L=4096, C=512] slice ROTATED so that its 2048 query rows are
always rows 0..2047 (attention and GN stats are invariant to key/value
ordering), which keeps the program SPMD-identical across cores.

On-core dataflow (fp8 DoubleRow matmuls for QKV projections and the
attention, bf16 elsewhere, fp32 accumulation):
  - GN is folded into the projections: A[c]=rstd[g]*gn_scale[c],
    B[c]=gn_bias[c]-mean[g]*A[c]; wq'=S*A (.) wq (row scaling),
    b'q = S*(B@wq + bq), with S=16 a power-of-2 pre-scale that keeps
    the fp8 weights out of the subnormal range. wo'=wo/S.
  - x is cast to bf16, transposed to xT [C, L] with PE transposes, and
    quantized to fp8 in the PSUM->SBUF copy. GN stats come from
    ones-vector matmuls over the bf16 stream (not the fp8 copy).
  - Q^T, K^T produced in fp8 [c, l] layout; V in fp8 [s, c] layout.
    All projection matmuls are fp8 DoubleRow (2 chained 256-deep MMs).
  - Scores: S^T[s, l] = (S*K)(S*Q)^T via DoubleRow; softmax's exp runs
    on ACT with scale=1/(S^2 sqrt(C)) and bias=-4 (a constant shift
    that cancels in the normalization but keeps exp outputs inside
    fp8's range: max score ~6.8 -> exp in [e^-11, e^3]). exp writes
    fp8 tiles directly; pairs of exp tiles form the [128,2,512]
    DoubleRow moving operand of the AV matmul.
  - Row sums Z come from ones-vector matmuls over DVE-pre-summed pairs
    of exp tiles; 1/Z is taken on a [128,4] column layout (cheap on
    DVE) after a PE transpose of the Z row.
  - O~^T = (S*V)^T A^T accumulated over s; the final projection uses
    wo/S (cancelling S) and is scaled by 1/Z per partition and fused
    with the bias + residual add in one DVE op, then DMA'd out.
"""

import sys

sys.path.insert(0, "/opt/trn_rl_repo")

import numpy as np

B, HH, WW, C = 4, 64, 64, 512
L = HH * WW          # 4096
G = 32               # groups
GS = C // G          # 16 channels per group
EPS = 1e-6
NCORES = 8
LQ = L // 2          # 2048 query rows per core
PT = 128             # partition tile
NT = L // PT         # 32 row tiles
NTQ = LQ // PT       # 16 query row tiles
CCH = C // PT        # 4 channel chunks
NB = 512             # matmul moving-free block
S = 16.0             # fp8 pre-scale folded into wq/wk/wv rows
SCALE = 1.0 / float(np.sqrt(C))
EXPB = -4.0          # constant shift inside exp (cancels in softmax)


def build_program():
    import concourse.bacc as bacc
    import concourse.bass as bass
    import concourse.mybir as mybir
    import concourse.tile as tile

    f32 = mybir.dt.float32
    bf16 = mybir.dt.bfloat16
    f8 = mybir.dt.float8e4
    AF = mybir.ActivationFunctionType
    DR = mybir.MatmulPerfMode.DoubleRow

    nc = bacc.Bacc(
        trn_type="TRN2",
        target_bir_lowering=False,
        debug=False,
        num_devices=NCORES,
    )

    x_d = nc.dram_tensor("x", [L, C], f32, kind="ExternalInput").ap()
    gs_d = nc.dram_tensor("gn_scale", [C], f32, kind="ExternalInput").ap()
    gb_d = nc.dram_tensor("gn_bias", [C], f32, kind="ExternalInput").ap()
    w_d = {}
    b_d = {}
    for n in "qkvo":
        w_d[n] = nc.dram_tensor("w" + n, [C, C], f32, kind="ExternalInput").ap()
        b_d[n] = nc.dram_tensor("b" + n, [C], f32, kind="ExternalInput").ap()
    eg_d = nc.dram_tensor("egrp", [G, C], f32, kind="ExternalInput").ap()
    eye_d = nc.dram_tensor("eye", [PT, PT], bf16, kind="ExternalInput").ap()
    y_d = nc.dram_tensor("y", [LQ, C], f32, kind="ExternalOutput").ap()

    with tile.TileContext(nc) as tc:
        with (
            tc.tile_pool(name="persist", bufs=1) as pp,
            tc.tile_pool(name="trans", bufs=1) as tp,
            tc.tile_pool(name="dram", bufs=1, space="DRAM") as dp,
            tc.tile_pool(name="psum", bufs=1, space="PSUM") as psp,
        ):
            # ---- constants ----
            ones_col = pp.tile([PT, 1], bf16, tag="ones_col")
            nc.vector.memset(ones_col, 1.0)
            ones_row = pp.tile([1, PT], bf16, tag="ones_row")
            nc.vector.memset(ones_row, 1.0)
            expb_t = pp.tile([PT, 1], f32, tag="expb")
            nc.vector.memset(expb_t, float(EXPB))

            # ---- persistent tensors ----
            xT = pp.tile([PT, CCH, L], f8, tag="xT")            # 2 MB
            qT = pp.tile([PT, CCH, LQ], f8, tag="qT")           # 1 MB
            kT = pp.tile([PT, CCH, L], f8, tag="kT")            # 2 MB
            v_sb = pp.tile([PT, NT, NB], f8, tag="v")           # 2 MB
            wb = {
                n: pp.tile([PT, CCH, C], f8, tag="wb" + n, name="wb_" + n)
                for n in "qkv"
            }
            wb["o"] = pp.tile([PT, CCH, C], bf16, tag="wbo", name="wb_o")

            one_f = pp.tile([1, 1], f32, tag="one_f")
            nc.vector.memset(one_f, 1.0)
            # group->channel indicator matrix [G, C] (host-provided constant)
            eg_sb = pp.tile([G, C], f32, tag="eg")
            nc.sync.dma_start(out=eg_sb, in_=eg_d)
            eye_sb = pp.tile([PT, PT], bf16, tag="eye")
            nc.sync.dma_start(out=eye_sb, in_=eye_d)

            def row_to_col(row, out_sb, nm):
                """[1, n*128] row -> [128, n] column layout, via K=1 matmuls."""
                n = out_sb.shape[-1]
                ps = psp.tile(
                    [PT, n], f32, tag="ps", bufs=2, name="r2c_" + nm
                )
                for m_ in range(n):
                    nc.tensor.matmul(
                        ps[:, m_ : m_ + 1],
                        row[0:1, m_ * PT : (m_ + 1) * PT],
                        one_f,
                        start=True,
                        stop=True,
                    )
                nc.vector.tensor_copy(out_sb, ps)

            # ---- small bias/scale loads (cheap, keep ahead of x) ----
            bq_row = pp.tile([1, C], f32, tag="bq_row")
            nc.sync.dma_start(out=bq_row, in_=b_d["q"].unsqueeze(0))
            bk_row = pp.tile([1, C], f32, tag="bk_row")
            nc.sync.dma_start(out=bk_row, in_=b_d["k"].unsqueeze(0))
            bv_row = pp.tile([1, C], f32, tag="bv_row")
            nc.sync.dma_start(out=bv_row, in_=b_d["v"].unsqueeze(0))
            bo_bc = pp.tile([PT, C], f32, tag="bo_bc")
            nc.gpsimd.dma_start(
                out=bo_bc, in_=b_d["o"].unsqueeze(0).broadcast_to([PT, C])
            )
            gs_row = pp.tile([1, C], f32, tag="gs_row")
            nc.sync.dma_start(out=gs_row, in_=gs_d.unsqueeze(0))
            gb_row = pp.tile([1, C], f32, tag="gb_row")
            nc.sync.dma_start(out=gb_row, in_=gb_d.unsqueeze(0))

            # ---- phase X: stream x, cast to bf16, stats, transpose ----
            sum_ps = psp.tile([1, C], f32, tag="pz")
            sq_ps = psp.tile([1, C], f32, tag="py")
            for t in range(NT):
                if t % 2 == 0:
                    g = t // 2
                    xf4 = tp.tile([PT, 2, C], f32, tag="xf4", bufs=2, name=f"xf4_{g}")
                    nc.sync.dma_start(
                        out=xf4,
                        in_=x_d[g * 2 * PT : (g + 1) * 2 * PT, :].rearrange(
                            "(i p) c -> p i c", p=PT
                        ),
                    )
                xf = xf4[:, t % 2, :]
                xb = tp.tile([PT, C], bf16, tag="xb", bufs=2)
                nc.vector.tensor_copy(xb, xf)
                sq = tp.tile([PT, C], bf16, tag="sq", bufs=2)
                nc.scalar.activation(out=sq, in_=xf, func=AF.Square)
                nc.tensor.matmul(
                    sum_ps, ones_col, xb, start=(t == 0), stop=(t == NT - 1)
                )
                nc.tensor.matmul(
                    sq_ps, ones_col, sq, start=(t == 0), stop=(t == NT - 1)
                )
                # transpose this row tile into xT via PE (4 blocks -> 1 bank);
                # the PSUM->SBUF copy quantizes to fp8
                t_ps = psp.tile([PT, NB], bf16, tag="ps", bufs=2, name=f"tps{t}")
                for j in range(CCH):
                    nc.tensor.transpose(
                        t_ps[:, j * PT : (j + 1) * PT],
                        xb[:, j * PT : (j + 1) * PT],
                        eye_sb,
                    )
                nc.vector.tensor_copy(
                    xT[:, :, t * PT : (t + 1) * PT], t_ps.rearrange("p (j i) -> p j i", j=CCH)
                )

            # ---- weight loads (issued after the x stream: x gets the DMA
            # ring first, weights trickle in during phase X) ----
            wf = {}
            for n in "qkvo":
                wf[n] = tp.tile([PT, CCH, C], f32, tag="wf", bufs=4, name="wf_" + n)
                nc.sync.dma_start(
                    out=wf[n], in_=w_d[n].rearrange("(j p) c -> p j c", p=PT)
                )

            # ---- phase S: GN stats -> A,B rows -> bounce to [128,4] ----
            s1 = tp.tile([1, G], f32, tag="small", bufs=8)
            nc.vector.reduce_sum(
                out=s1,
                in_=sum_ps.rearrange("p (g d) -> p g d", g=G),
                axis=mybir.AxisListType.X,
            )
            s2 = tp.tile([1, G], f32, tag="small", bufs=8)
            nc.vector.reduce_sum(
                out=s2,
                in_=sq_ps.rearrange("p (g d) -> p g d", g=G),
                axis=mybir.AxisListType.X,
            )
            inv_n = 1.0 / float(L * GS)
            mean = tp.tile([1, G], f32, tag="small", bufs=8)
            nc.vector.tensor_scalar_mul(mean, s1, inv_n)
            ex2 = tp.tile([1, G], f32, tag="small", bufs=8)
            nc.vector.tensor_scalar_mul(ex2, s2, inv_n)
            m2 = tp.tile([1, G], f32, tag="small", bufs=8)
            nc.vector.tensor_mul(m2, mean, mean)
            var = tp.tile([1, G], f32, tag="small", bufs=8)
            nc.vector.tensor_sub(var, ex2, m2)
            sd = tp.tile([1, G], f32, tag="small", bufs=8)
            eps_t = tp.tile([1, 1], f32, tag="small", bufs=8)
            nc.vector.memset(eps_t, float(EPS))
            nc.scalar.activation(out=sd, in_=var, func=AF.Sqrt, bias=eps_t)
            rstd = tp.tile([1, G], f32, tag="small", bufs=8)
            nc.vector.reciprocal(rstd, sd)

            # mean/rstd [1,32] -> columns [32,1] -> expand to channel rows
            gcol_ps = psp.tile([G, 2], f32, tag="ps", bufs=2)
            nc.tensor.matmul(gcol_ps[:, 0:1], rstd, one_f, start=True, stop=True)
            nc.tensor.matmul(gcol_ps[:, 1:2], mean, one_f, start=True, stop=True)
            gcol = tp.tile([G, 2], f32, tag="small", bufs=8)
            nc.vector.tensor_copy(gcol, gcol_ps)
            rstd_e_ps = psp.tile([1, C], f32, tag="ps", bufs=2)
            nc.tensor.matmul(rstd_e_ps, gcol[:, 0:1], eg_sb, start=True, stop=True)
            a_row = tp.tile([1, C], f32, tag="row", bufs=4)
            nc.vector.tensor_mul(a_row, rstd_e_ps, gs_row)
            mean_e_ps = psp.tile([1, C], f32, tag="ps", bufs=2)
            nc.tensor.matmul(mean_e_ps, gcol[:, 1:2], eg_sb, start=True, stop=True)
            mb = tp.tile([1, C], f32, tag="row", bufs=4)
            nc.vector.tensor_mul(mb, mean_e_ps, a_row)
            b_row = tp.tile([1, C], f32, tag="row", bufs=4)
            nc.vector.tensor_sub(b_row, gb_row, mb)
            # S-scaled A column (folded into wq/wk/wv rows)
            a16_row = tp.tile([1, C], f32, tag="row", bufs=4)
            nc.vector.tensor_scalar_mul(a16_row, a_row, float(S))
            aT = pp.tile([PT, CCH], f32, tag="aT")
            row_to_col(a16_row, aT, "aT")
            bT = pp.tile([PT, CCH], f32, tag="bT")
            row_to_col(b_row, bT, "bT")

            # ---- phase WP: fold GN into weights & biases ----
            # b'q/b'k = S*(B @ w + b), computed as rows then moved to columns
            bq_f = pp.tile([PT, CCH], f32, tag="bq_f")
            bk_f = pp.tile([PT, CCH], f32, tag="bk_f")
            for n, bias_row, out_t in (("q", bq_row, bq_f), ("k", bk_row, bk_f)):
                psb = psp.tile([1, C], f32, tag="ps", bufs=2, name="psb_" + n)
                for j in range(CCH):
                    nc.tensor.matmul(
                        psb,
                        bT[:, j : j + 1],
                        wf[n][:, j, :],
                        start=(j == 0),
                        stop=(j == CCH - 1),
                    )
                bp_row = tp.tile([1, C], f32, tag="row", bufs=4, name="bp_" + n)
                nc.vector.tensor_add(bp_row, psb, bias_row)
                bp16_row = tp.tile([1, C], f32, tag="row", bufs=4, name="bp16_" + n)
                nc.vector.tensor_scalar_mul(bp16_row, bp_row, float(S))
                row_to_col(bp16_row, out_t, "b" + n)
            # b'v as a row [1, 512] (bias enters V via ones-row matmul)
            psv = psp.tile([1, C], f32, tag="pz")
            for j in range(CCH):
                nc.tensor.matmul(
                    psv,
                    bT[:, j : j + 1],
                    wf["v"][:, j, :],
                    start=(j == 0),
                    stop=(j == CCH - 1),
                )
            bvp = tp.tile([1, C], f32, tag="row", bufs=4)
            nc.vector.tensor_add(bvp, psv, bv_row)
            bvp16 = tp.tile([1, C], f32, tag="row", bufs=4)
            nc.vector.tensor_scalar_mul(bvp16, bvp, float(S))
            bvp_bf = pp.tile([1, C], bf16, tag="bvp_bf")
            nc.vector.tensor_copy(bvp_bf, bvp16)
            # b'v broadcast across partitions (for DVE bias add in V copies)
            bvbc_ps = psp.tile([PT, C], f32, tag="ps", bufs=2)
            nc.tensor.matmul(bvbc_ps, ones_row, bvp_bf, start=True, stop=True)
            bvbc = pp.tile([PT, C], f32, tag="bvbc")
            nc.vector.tensor_copy(bvbc, bvbc_ps)

            # scale+cast weights: wq/k/v rows scaled by S*A (per input channel)
            for n in "qkv":
                for j in range(CCH):
                    nc.vector.tensor_scalar_mul(
                        wb[n][:, j, :], wf[n][:, j, :], aT[:, j : j + 1]
                    )
            for j in range(CCH):
                nc.vector.tensor_scalar_mul(
                    wb["o"][:, j, :], wf["o"][:, j, :], 1.0 / float(S)
                )

            # ---- phase P: projections (fp8 DoubleRow, 2x256-deep chains) ----
            # Q^T [c, l] for query rows 0..2047
            for m in range(CCH):
                for lb in range(LQ // NB):
                    ps = psp.tile([PT, NB], f32, tag="po", bufs=4)
                    for jp in range(CCH // 2):
                        nc.tensor.matmul(
                            ps,
                            wb["q"][:, 2 * jp : 2 * jp + 2, m * PT : (m + 1) * PT],
                            xT[:, 2 * jp : 2 * jp + 2, lb * NB : (lb + 1) * NB],
                            start=(jp == 0),
                            stop=(jp == CCH // 2 - 1),
                            perf_mode=DR,
                        )
                    nc.vector.tensor_scalar_add(
                        qT[:, m, lb * NB : (lb + 1) * NB], ps, bq_f[:, m : m + 1]
                    )
            # K^T [c, l] for all rows
            for m in range(CCH):
                for lb in range(L // NB):
                    ps = psp.tile([PT, NB], f32, tag="po", bufs=4)
                    for jp in range(CCH // 2):
                        nc.tensor.matmul(
                            ps,
                            wb["k"][:, 2 * jp : 2 * jp + 2, m * PT : (m + 1) * PT],
                            xT[:, 2 * jp : 2 * jp + 2, lb * NB : (lb + 1) * NB],
                            start=(jp == 0),
                            stop=(jp == CCH // 2 - 1),
                            perf_mode=DR,
                        )
                    nc.vector.tensor_scalar_add(
                        kT[:, m, lb * NB : (lb + 1) * NB], ps, bk_f[:, m : m + 1]
                    )
            # V natural [s, c] for all rows; bias via broadcast add
            for t in range(NT):
                ps = psp.tile([PT, NB], f32, tag="po", bufs=4)
                for jp in range(CCH // 2):
                    nc.tensor.matmul(
                        ps,
                        xT[:, 2 * jp : 2 * jp + 2, t * PT : (t + 1) * PT],
                        wb["v"][:, 2 * jp : 2 * jp + 2, :],
                        start=(jp == 0),
                        stop=(jp == CCH // 2 - 1),
                        perf_mode=DR,
                    )
                nc.vector.tensor_add(v_sb[:, t, :], ps, bvbc)

            # ---- phase A: attention, one 512-wide query block at a time ----
            for lb in range(LQ // NB):
                zps = psp.tile([1, NB], f32, tag="pz")
                ops = [
                    psp.tile([PT, NB], f32, tag="po", bufs=4, name=f"ops{m}")
                    for m in range(CCH)
                ]
                zp_group = []
                for st2 in range(NT // 2):
                    a_pair = tp.tile([PT, 2, NB], f8, tag="a_t", bufs=3)
                    for half in range(2):
                        st = 2 * st2 + half
                        sps = psp.tile([PT, NB], f32, tag="ps", bufs=2)
                        for jp in range(CCH // 2):
                            nc.tensor.matmul(
                                sps,
                                kT[:, 2 * jp : 2 * jp + 2, st * PT : (st + 1) * PT],
                                qT[:, 2 * jp : 2 * jp + 2, lb * NB : (lb + 1) * NB],
                                start=(jp == 0),
                                stop=(jp == CCH // 2 - 1),
                                perf_mode=DR,
                            )
                        nc.scalar.activation(
                            out=a_pair[:, half, :],
                            in_=sps,
                            func=AF.Exp,
                            scale=SCALE / float(S * S),
                            bias=expb_t,
                        )
                    for m in range(CCH):
                        nc.tensor.matmul(
                            ops[m],
                            v_sb[:, 2 * st2 : 2 * st2 + 2, m * PT : (m + 1) * PT],
                            a_pair,
                            start=(st2 == 0),
                            stop=(st2 == NT // 2 - 1),
                            perf_mode=DR,
                        )
                    # Z: pre-sum exp-tile pairs on DVE, 1 matmul per 4 tiles
                    zp = tp.tile([PT, NB], bf16, tag="zp", bufs=2, name=f"zp{st2 % 2}")
                    nc.vector.tensor_add(zp, a_pair[:, 0, :], a_pair[:, 1, :])
                    zp_group.append(zp)
                    if len(zp_group) == 2:
                        g = st2 // 2
                        z4 = tp.tile([PT, NB], bf16, tag="z4", bufs=2)
                        nc.vector.tensor_add(z4, zp_group[0], zp_group[1])
                        nc.tensor.matmul(
                            zps,
                            ones_col,
                            z4,
                            start=(g == 0),
                            stop=(g == NT // 4 - 1),
                        )
                        zp_group = []
                # Z row -> [128, 4] columns, then cheap per-partition 1/Z
                zrow = tp.tile([1, NB], f32, tag="row", bufs=4, name=f"zrow{lb}")
                nc.vector.tensor_copy(zrow, zps)
                zTr = tp.tile([PT, NB // PT], f32, tag="zTr", bufs=2)
                row_to_col(zrow, zTr, f"zT{lb}")
                zT = tp.tile([PT, NB // PT], f32, tag="zT", bufs=2)
                nc.vector.reciprocal(zT, zTr)
                obf = []
                for m in range(CCH):
                    ot = tp.tile([PT, NB], bf16, tag="obf", bufs=4, name=f"obf{m}")
                    nc.vector.tensor_copy(ot, ops[m])
                    obf.append(ot)
                # final projection; normalize by 1/Z and add bias+residual
                xr4 = tp.tile([PT, 4, C], f32, tag="xr4", bufs=1, name=f"xr4_{lb}")
                nc.sync.dma_start(
                    out=xr4,
                    in_=x_d[lb * NB : (lb + 1) * NB, :].rearrange(
                        "(i p) c -> p i c", p=PT
                    ),
                )
                for sub in range(NB // PT):
                    t = lb * (NB // PT) + sub
                    yps = psp.tile([PT, NB], f32, tag="py", bufs=1)
                    for m in range(CCH):
                        nc.tensor.matmul(
                            yps,
                            obf[m][:, sub * PT : (sub + 1) * PT],
                            wb["o"][:, m, :],
                            start=(m == 0),
                            stop=(m == CCH - 1),
                        )
                    xrb = tp.tile([PT, C], f32, tag="xrb", bufs=1)
                    nc.vector.tensor_add(xrb, xr4[:, sub, :], bo_bc)
                    yt = tp.tile([PT, C], f32, tag="yt", bufs=2)
                    nc.vector.scalar_tensor_tensor(
                        out=yt,
                        in0=yps,
                        scalar=zT[:, sub : sub + 1],
                        in1=xrb,
                        op0=mybir.AluOpType.mult,
                        op1=mybir.AluOpType.add,
                    )
                    nc.sync.dma_start(out=y_d[t * PT : (t + 1) * PT, :], in_=yt)

    nc.compile()
    return nc


_NC_CACHE = None


def _get_program():
    global _NC_CACHE
    if _NC_CACHE is None:
        _NC_CACHE = build_program()
    return _NC_CACHE


def make_in_maps(inputs):
    hs = np.ascontiguousarray(np.asarray(inputs["hidden_states"], np.float32))
    ws = {n: np.ascontiguousarray(np.asarray(inputs["w" + n], np.float32)) for n in "qkvo"}
    bs = {n: np.ascontiguousarray(np.asarray(inputs["b" + n], np.float32)) for n in "qkvo"}
    gsc = np.ascontiguousarray(np.asarray(inputs["gn_scale"], np.float32))
    gbi = np.ascontiguousarray(np.asarray(inputs["gn_bias"], np.float32))
    import ml_dtypes
    eye = np.eye(PT, dtype=ml_dtypes.bfloat16)
    eg = np.zeros((G, C), np.float32)
    eg[np.arange(C) // GS, np.arange(C)] = 1.0
    in_maps = []
    for core in range(NCORES):
        b, h = core // 2, core % 2
        xb = hs[b].reshape(L, C)
        x_roll = np.ascontiguousarray(np.roll(xb, -h * LQ, axis=0))
        m = {"x": x_roll, "gn_scale": gsc, "gn_bias": gbi, "egrp": eg, "eye": eye}
        for n in "qkvo":
            m["w" + n] = ws[n]
            m["b" + n] = bs[n]
        in_maps.append(m)
    return in_maps


def assemble(results):
    out = np.empty((B, L, C), np.float32)
    for core in range(NCORES):
        b, h = core // 2, core % 2
        out[b, h * LQ : (h + 1) * LQ] = results[core]["y"]
    return out.reshape(B, HH, WW, C)


def kernel(**inputs):
    from concourse.bass_utils import run_bass_kernel_spmd

    nc = _get_program()
    in_maps = make_in_maps(inputs)
    res = run_bass_kernel_spmd(nc, in_maps, list(range(NCORES)))
    return assemble(res.results)


if __name__ == "__main__":
    rng = np.random.default_rng(0)
    s = 1.0 / np.sqrt(C)
    inputs = {
        "hidden_states": rng.standard_normal((B, HH, WW, C), np.float32),
        "gn_scale": np.ones(C, np.float32),
        "gn_bias": np.zeros(C, np.float32),
    }
    for n in "qkvo":
        inputs["w" + n] = (rng.standard_normal((C, C)) * s).astype(np.float32)
        inputs["b" + n] = np.zeros(C, np.float32)
    out = kernel(**inputs)
    print(out.shape, out.dtype)


# revision 15
# speedup vs baseline: 1.0129x; 1.0129x over previous
"""Trainium2 Bass kernel for GroupNorm + single-head self-attention block.

Reference computation (per batch b):
    xn = GroupNorm(x; 32 groups over (L, C/32)) * gn_scale + gn_bias
    q, k, v = xn@wq+bq, xn@wk+bk, xn@wv+bv
    out = softmax(q k^T / sqrt(C)) v @ wo + bo + x

Sharding: 8 cores = 4 batches x 2 query-halves. Each core receives its
batch's [L=4096, C=512] slice ROTATED so that its 2048 query rows are
always rows 0..2047 (attention and GN stats are invariant to key/value
ordering), which keeps the program SPMD-identical across cores.

On-core dataflow (fp8 DoubleRow matmuls for QKV projections and the
attention, bf16 elsewhere, fp32 accumulation):
  - GN is folded into the projections: A[c]=rstd[g]*gn_scale[c],
    B[c]=gn_bias[c]-mean[g]*A[c]; wq'=S*A (.) wq (row scaling),
    b'q = S*(B@wq + bq), with S=16 a power-of-2 pre-scale that keeps
    the fp8 weights out of the subnormal range. wo'=wo/S.
  - x is cast to bf16, transposed to xT [C, L] with PE transposes, and
    quantized to fp8 in the PSUM->SBUF copy. GN stats come from
    ones-vector matmuls over the bf16 stream (not the fp8 copy).
  - Q^T, K^T produced in fp8 [c, l] layout; V in fp8 [s, c] layout.
    All projection matmuls are fp8 DoubleRow (2 chained 256-deep MMs).
  - Scores: S^T[s, l] = (S*K)(S*Q)^T via DoubleRow; softmax's exp runs
    on ACT with scale=1/(S^2 sqrt(C)) and bias=-4 (a constant shift
    that cancels in the normalization but keeps exp outputs inside
    fp8's range: max score ~6.8 -> exp in [e^-11, e^3]). exp writes
    fp8 tiles directly; pairs of exp tiles form the [128,2,512]
    DoubleRow moving operand of the AV matmul.
  - Row sums Z come from ones-vector matmuls over DVE-pre-summed pairs
    of exp tiles; 1/Z is taken on a [128,4] column layout (cheap on
    DVE) after a PE transpose of the Z row.
  - O~^T = (S*V)^T A^T accumulated over s; the final projection uses
    wo/S (cancelling S) and is scaled by 1/Z per partition and fused
    with the bias + residual add in one DVE op, then DMA'd out.
"""

import sys

sys.path.insert(0, "/opt/trn_rl_repo")

import numpy as np

B, HH, WW, C = 4, 64, 64, 512
L = HH * WW          # 4096
G = 32               # groups
GS = C // G          # 16 channels per group
EPS = 1e-6
NCORES = 8
LQ = L // 2          # 2048 query rows per core
PT = 128             # partition tile
NT = L // PT         # 32 row tiles
NTQ = LQ // PT       # 16 query row tiles
CCH = C // PT        # 4 channel chunks
NB = 512             # matmul moving-free block
S = 16.0             # fp8 pre-scale folded into wq/wk/wv rows
SCALE = 1.0 / float(np.sqrt(C))
EXPB = -4.0          # constant shift inside exp (cancels in softmax)


def build_program():
    import concourse.bacc as bacc
    import concourse.bass as bass
    import concourse.mybir as mybir
    import concourse.tile as tile

    f32 = mybir.dt.float32
    bf16 = mybir.dt.bfloat16
    f8 = mybir.dt.float8e4
    AF = mybir.ActivationFunctionType
    DR = mybir.MatmulPerfMode.DoubleRow

    nc = bacc.Bacc(
        trn_type="TRN2",
        target_bir_lowering=False,
        debug=False,
        num_devices=NCORES,
    )

    x_d = nc.dram_tensor("x", [L, C], f32, kind="ExternalInput").ap()
    gs_d = nc.dram_tensor("gn_scale", [C], f32, kind="ExternalInput").ap()
    gb_d = nc.dram_tensor("gn_bias", [C], f32, kind="ExternalInput").ap()
    w_d = {}
    b_d = {}
    for n in "qkvo":
        w_d[n] = nc.dram_tensor("w" + n, [C, C], f32, kind="ExternalInput").ap()
        b_d[n] = nc.dram_tensor("b" + n, [C], f32, kind="ExternalInput").ap()
    eg_d = nc.dram_tensor("egrp", [G, C], f32, kind="ExternalInput").ap()
    eye_d = nc.dram_tensor("eye", [PT, PT], bf16, kind="ExternalInput").ap()
    y_d = nc.dram_tensor("y", [LQ, C], f32, kind="ExternalOutput").ap()

    with tile.TileContext(nc) as tc:
        with (
            tc.tile_pool(name="persist", bufs=1) as pp,
            tc.tile_pool(name="trans", bufs=1) as tp,
            tc.tile_pool(name="dram", bufs=1, space="DRAM") as dp,
            tc.tile_pool(name="psum", bufs=1, space="PSUM") as psp,
        ):
            # ---- constants ----
            ones_col = pp.tile([PT, 1], bf16, tag="ones_col")
            nc.vector.memset(ones_col, 1.0)
            ones_row = pp.tile([1, PT], bf16, tag="ones_row")
            nc.vector.memset(ones_row, 1.0)
            expb_t = pp.tile([PT, 1], f32, tag="expb")
            nc.vector.memset(expb_t, float(EXPB))

            # ---- persistent tensors ----
            xT = pp.tile([PT, CCH, L], f8, tag="xT")            # 2 MB
            qT = pp.tile([PT, CCH, LQ], f8, tag="qT")           # 1 MB
            kT = pp.tile([PT, CCH, L], f8, tag="kT")            # 2 MB
            v_sb = pp.tile([PT, NT, NB], f8, tag="v")           # 2 MB
            wb = {
                n: pp.tile([PT, CCH, C], f8, tag="wb" + n, name="wb_" + n)
                for n in "qkv"
            }
            wb["o"] = pp.tile([PT, CCH, C], f8, tag="wbo", name="wb_o")

            one_f = pp.tile([1, 1], f32, tag="one_f")
            nc.vector.memset(one_f, 1.0)
            # eye is the only DMA the x stream waits behind on the sync ring
            eye_sb = pp.tile([PT, PT], bf16, tag="eye")
            nc.sync.dma_start(out=eye_sb, in_=eye_d)
            # group->channel indicator matrix [G, C] (host-provided constant)
            eg_sb = pp.tile([G, C], f32, tag="eg")
            nc.gpsimd.dma_start(out=eg_sb, in_=eg_d)

            def row_to_col(row, out_sb, nm):
                """[1, n*128] row -> [128, n] column layout, via K=1 matmuls."""
                n = out_sb.shape[-1]
                ps = psp.tile(
                    [PT, n], f32, tag="ps", bufs=2, name="r2c_" + nm
                )
                for m_ in range(n):
                    nc.tensor.matmul(
                        ps[:, m_ : m_ + 1],
                        row[0:1, m_ * PT : (m_ + 1) * PT],
                        one_f,
                        start=True,
                        stop=True,
                    )
                nc.vector.tensor_copy(out_sb, ps)

            # ---- small bias/scale loads (gpsimd ring; off the x stream) ----
            bq_row = pp.tile([1, C], f32, tag="bq_row")
            nc.gpsimd.dma_start(out=bq_row, in_=b_d["q"].unsqueeze(0))
            bk_row = pp.tile([1, C], f32, tag="bk_row")
            nc.gpsimd.dma_start(out=bk_row, in_=b_d["k"].unsqueeze(0))
            bv_row = pp.tile([1, C], f32, tag="bv_row")
            nc.gpsimd.dma_start(out=bv_row, in_=b_d["v"].unsqueeze(0))
            bo_bc = pp.tile([PT, C], f32, tag="bo_bc")
            nc.gpsimd.dma_start(
                out=bo_bc, in_=b_d["o"].unsqueeze(0).broadcast_to([PT, C])
            )
            gs_row = pp.tile([1, C], f32, tag="gs_row")
            nc.gpsimd.dma_start(out=gs_row, in_=gs_d.unsqueeze(0))
            gb_row = pp.tile([1, C], f32, tag="gb_row")
            nc.gpsimd.dma_start(out=gb_row, in_=gb_d.unsqueeze(0))
            # ---- weight loads (gpsimd ring, concurrent with the x stream) ----
            wf = {}
            for n in "qkvo":
                wf[n] = tp.tile([PT, CCH, C], f32, tag="wf", bufs=4, name="wf_" + n)
                nc.gpsimd.dma_start(
                    out=wf[n], in_=w_d[n].rearrange("(j p) c -> p j c", p=PT)
                )

            # ---- phase X: stream x, cast to bf16, stats, transpose ----
            sum_ps = psp.tile([1, C], f32, tag="pz")
            sq_ps = psp.tile([1, C], f32, tag="py")
            for t in range(NT):
                if t % 2 == 0:
                    g = t // 2
                    xf4 = tp.tile([PT, 2, C], f32, tag="xf4", bufs=4, name=f"xf4_{g}")
                    nc.sync.dma_start(
                        out=xf4,
                        in_=x_d[g * 2 * PT : (g + 1) * 2 * PT, :].rearrange(
                            "(i p) c -> p i c", p=PT
                        ),
                    )
                xf = xf4[:, t % 2, :]
                xb = tp.tile([PT, C], bf16, tag="xb", bufs=2)
                nc.vector.tensor_copy(xb, xf)
                sq = tp.tile([PT, C], bf16, tag="sq", bufs=2)
                nc.scalar.activation(out=sq, in_=xf, func=AF.Square)
                nc.tensor.matmul(
                    sum_ps, ones_col, xb, start=(t == 0), stop=(t == NT - 1)
                )
                nc.tensor.matmul(
                    sq_ps, ones_col, sq, start=(t == 0), stop=(t == NT - 1)
                )
                # transpose this row tile into xT via PE (4 blocks -> 1 bank);
                # the PSUM->SBUF copy quantizes to fp8
                t_ps = psp.tile([PT, NB], bf16, tag="ps", bufs=2, name=f"tps{t}")
                for j in range(CCH):
                    nc.tensor.transpose(
                        t_ps[:, j * PT : (j + 1) * PT],
                        xb[:, j * PT : (j + 1) * PT],
                        eye_sb,
                    )
                nc.vector.tensor_copy(
                    xT[:, :, t * PT : (t + 1) * PT], t_ps.rearrange("p (j i) -> p j i", j=CCH)
                )

            # bf16 copies of wq/wk/wv for the (tiny) B@w bias-fold matmuls —
            # bf16 matmuls run 4x faster than fp32 ones
            wfb = {}
            for n in "qkv":
                wfb[n] = tp.tile([PT, CCH, C], bf16, tag="wfb", bufs=3, name="wfb_" + n)
                for j in range(CCH):
                    nc.vector.tensor_copy(wfb[n][:, j, :], wf[n][:, j, :])

            # ---- phase S: GN stats -> A,B rows -> bounce to [128,4] ----
            s1 = tp.tile([1, G], f32, tag="small", bufs=8)
            nc.vector.reduce_sum(
                out=s1,
                in_=sum_ps.rearrange("p (g d) -> p g d", g=G),
                axis=mybir.AxisListType.X,
            )
            s2 = tp.tile([1, G], f32, tag="small", bufs=8)
            nc.vector.reduce_sum(
                out=s2,
                in_=sq_ps.rearrange("p (g d) -> p g d", g=G),
                axis=mybir.AxisListType.X,
            )
            inv_n = 1.0 / float(L * GS)
            mean = tp.tile([1, G], f32, tag="small", bufs=8)
            nc.vector.tensor_scalar_mul(mean, s1, inv_n)
            ex2 = tp.tile([1, G], f32, tag="small", bufs=8)
            nc.vector.tensor_scalar_mul(ex2, s2, inv_n)
            m2 = tp.tile([1, G], f32, tag="small", bufs=8)
            nc.vector.tensor_mul(m2, mean, mean)
            var = tp.tile([1, G], f32, tag="small", bufs=8)
            nc.vector.tensor_sub(var, ex2, m2)
            sd = tp.tile([1, G], f32, tag="small", bufs=8)
            eps_t = tp.tile([1, 1], f32, tag="small", bufs=8)
            nc.vector.memset(eps_t, float(EPS))
            nc.scalar.activation(out=sd, in_=var, func=AF.Sqrt, bias=eps_t)
            rstd = tp.tile([1, G], f32, tag="small", bufs=8)
            nc.vector.reciprocal(rstd, sd)

            # mean/rstd [1,32] -> columns [32,1] -> expand to channel rows
            gcol_ps = psp.tile([G, 2], f32, tag="ps", bufs=2)
            nc.tensor.matmul(gcol_ps[:, 0:1], rstd, one_f, start=True, stop=True)
            nc.tensor.matmul(gcol_ps[:, 1:2], mean, one_f, start=True, stop=True)
            gcol = tp.tile([G, 2], f32, tag="small", bufs=8)
            nc.vector.tensor_copy(gcol, gcol_ps)
            rstd_e_ps = psp.tile([1, C], f32, tag="ps", bufs=2)
            nc.tensor.matmul(rstd_e_ps, gcol[:, 0:1], eg_sb, start=True, stop=True)
            a_row = tp.tile([1, C], f32, tag="row", bufs=4)
            nc.vector.tensor_mul(a_row, rstd_e_ps, gs_row)
            mean_e_ps = psp.tile([1, C], f32, tag="ps", bufs=2)
            nc.tensor.matmul(mean_e_ps, gcol[:, 1:2], eg_sb, start=True, stop=True)
            mb = tp.tile([1, C], f32, tag="row", bufs=4)
            nc.vector.tensor_mul(mb, mean_e_ps, a_row)
            b_row = tp.tile([1, C], f32, tag="row", bufs=4)
            nc.vector.tensor_sub(b_row, gb_row, mb)
            # S-scaled A column (folded into wq/wk/wv rows)
            a16_row = tp.tile([1, C], f32, tag="row", bufs=4)
            nc.vector.tensor_scalar_mul(a16_row, a_row, float(S))
            aT = pp.tile([PT, CCH], f32, tag="aT")
            row_to_col(a16_row, aT, "aT")
            bT = pp.tile([PT, CCH], f32, tag="bT")
            row_to_col(b_row, bT, "bT")
            bT_bf = pp.tile([PT, CCH], bf16, tag="bT_bf")
            nc.vector.tensor_copy(bT_bf, bT)

            # ---- phase WP: fold GN into weights & biases ----
            # b'q/b'k = S*(B @ w + b), computed as rows then moved to columns
            bq_f = pp.tile([PT, CCH], f32, tag="bq_f")
            bk_f = pp.tile([PT, CCH], f32, tag="bk_f")
            for n, bias_row, out_t in (("q", bq_row, bq_f), ("k", bk_row, bk_f)):
                psb = psp.tile([1, C], f32, tag="ps", bufs=2, name="psb_" + n)
                for j in range(CCH):
                    nc.tensor.matmul(
                        psb,
                        bT_bf[:, j : j + 1],
                        wfb[n][:, j, :],
                        start=(j == 0),
                        stop=(j == CCH - 1),
                    )
                bp_row = tp.tile([1, C], f32, tag="row", bufs=4, name="bp_" + n)
                nc.vector.tensor_add(bp_row, psb, bias_row)
                bp16_row = tp.tile([1, C], f32, tag="row", bufs=4, name="bp16_" + n)
                nc.vector.tensor_scalar_mul(bp16_row, bp_row, float(S))
                row_to_col(bp16_row, out_t, "b" + n)
            # b'v as a row [1, 512] (bias enters V via ones-row matmul)
            psv = psp.tile([1, C], f32, tag="pz")
            for j in range(CCH):
                nc.tensor.matmul(
                    psv,
                    bT_bf[:, j : j + 1],
                    wfb["v"][:, j, :],
                    start=(j == 0),
                    stop=(j == CCH - 1),
                )
            bvp = tp.tile([1, C], f32, tag="row", bufs=4)
            nc.vector.tensor_add(bvp, psv, bv_row)
            bvp16 = tp.tile([1, C], f32, tag="row", bufs=4)
            nc.vector.tensor_scalar_mul(bvp16, bvp, float(S))
            bvp_bf = pp.tile([1, C], bf16, tag="bvp_bf")
            nc.vector.tensor_copy(bvp_bf, bvp16)
            # b'v broadcast across partitions (for DVE bias add in V copies)
            bvbc_ps = psp.tile([PT, C], f32, tag="ps", bufs=2)
            nc.tensor.matmul(bvbc_ps, ones_row, bvp_bf, start=True, stop=True)
            bvbc = pp.tile([PT, C], f32, tag="bvbc")
            nc.vector.tensor_copy(bvbc, bvbc_ps)

            # scale+cast weights: wq/k/v rows scaled by S*A (per input channel)
            for n in "qkv":
                for j in range(CCH):
                    nc.vector.tensor_scalar_mul(
                        wb[n][:, j, :], wf[n][:, j, :], aT[:, j : j + 1]
                    )
            # wo in fp8 scaled by 64/S: the O~ accumulator is scaled by 1/64
            # in the PSUM->fp8 copy, so the product keeps the same scale
            for j in range(CCH):
                nc.vector.tensor_scalar_mul(
                    wb["o"][:, j, :], wf["o"][:, j, :], 64.0 / float(S)
                )

            # ---- phase P: projections (fp8 DoubleRow, 2x256-deep chains) ----
            # Q^T [c, l] for query rows 0..2047
            for m in range(CCH):
                for lb in range(LQ // NB):
                    ps = psp.tile([PT, NB], f32, tag="po", bufs=4)
                    for jp in range(CCH // 2):
                        nc.tensor.matmul(
                            ps,
                            wb["q"][:, 2 * jp : 2 * jp + 2, m * PT : (m + 1) * PT],
                            xT[:, 2 * jp : 2 * jp + 2, lb * NB : (lb + 1) * NB],
                            start=(jp == 0),
                            stop=(jp == CCH // 2 - 1),
                            perf_mode=DR,
                        )
                    nc.vector.tensor_scalar_add(
                        qT[:, m, lb * NB : (lb + 1) * NB], ps, bq_f[:, m : m + 1]
                    )
            # K^T [c, l] for all rows
            for m in range(CCH):
                for lb in range(L // NB):
                    ps = psp.tile([PT, NB], f32, tag="po", bufs=4)
                    for jp in range(CCH // 2):
                        nc.tensor.matmul(
                            ps,
                            wb["k"][:, 2 * jp : 2 * jp + 2, m * PT : (m + 1) * PT],
                            xT[:, 2 * jp : 2 * jp + 2, lb * NB : (lb + 1) * NB],
                            start=(jp == 0),
                            stop=(jp == CCH // 2 - 1),
                            perf_mode=DR,
                        )
                    nc.vector.tensor_scalar_add(
                        kT[:, m, lb * NB : (lb + 1) * NB], ps, bk_f[:, m : m + 1]
                    )
            # V natural [s, c] for all rows; bias via broadcast add
            for t in range(NT):
                ps = psp.tile([PT, NB], f32, tag="po", bufs=4)
                for jp in range(CCH // 2):
                    nc.tensor.matmul(
                        ps,
                        xT[:, 2 * jp : 2 * jp + 2, t * PT : (t + 1) * PT],
                        wb["v"][:, 2 * jp : 2 * jp + 2, :],
                        start=(jp == 0),
                        stop=(jp == CCH // 2 - 1),
                        perf_mode=DR,
                    )
                nc.vector.tensor_add(v_sb[:, t, :], ps, bvbc)

            # ---- phase A: attention, one 512-wide query block at a time ----
            for lb in range(LQ // NB):
                zps = psp.tile([1, NB], f32, tag="pz")
                ops = [
                    psp.tile([PT, NB], f32, tag="po", bufs=4, name=f"ops{m}")
                    for m in range(CCH)
                ]
                # prefetch the residual rows for this block's epilogue
                xr4 = tp.tile([PT, 4, C], f32, tag="xr4", bufs=2, name=f"xr4_{lb}")
                nc.sync.dma_start(
                    out=xr4,
                    in_=x_d[lb * NB : (lb + 1) * NB, :].rearrange(
                        "(i p) c -> p i c", p=PT
                    ),
                )
                zp_group = []
                for st2 in range(NT // 2):
                    a_pair = tp.tile([PT, 2, NB], f8, tag="a_t", bufs=3)
                    for half in range(2):
                        st = 2 * st2 + half
                        sps = psp.tile([PT, NB], f32, tag="ps", bufs=2)
                        for jp in range(CCH // 2):
                            nc.tensor.matmul(
                                sps,
                                kT[:, 2 * jp : 2 * jp + 2, st * PT : (st + 1) * PT],
                                qT[:, 2 * jp : 2 * jp + 2, lb * NB : (lb + 1) * NB],
                                start=(jp == 0),
                                stop=(jp == CCH // 2 - 1),
                                perf_mode=DR,
                            )
                        nc.scalar.activation(
                            out=a_pair[:, half, :],
                            in_=sps,
                            func=AF.Exp,
                            scale=SCALE / float(S * S),
                            bias=expb_t,
                        )
                    for m in range(CCH):
                        nc.tensor.matmul(
                            ops[m],
                            v_sb[:, 2 * st2 : 2 * st2 + 2, m * PT : (m + 1) * PT],
                            a_pair,
                            start=(st2 == 0),
                            stop=(st2 == NT // 2 - 1),
                            perf_mode=DR,
                        )
                    # Z: pre-sum exp-tile pairs on DVE, 1 matmul per 4 tiles
                    zp = tp.tile([PT, NB], bf16, tag="zp", bufs=2, name=f"zp{st2 % 2}")
                    nc.vector.tensor_add(zp, a_pair[:, 0, :], a_pair[:, 1, :])
                    zp_group.append(zp)
                    if len(zp_group) == 2:
                        g = st2 // 2
                        z4 = tp.tile([PT, NB], bf16, tag="z4", bufs=2)
                        nc.vector.tensor_add(z4, zp_group[0], zp_group[1])
                        nc.tensor.matmul(
                            zps,
                            ones_col,
                            z4,
                            start=(g == 0),
                            stop=(g == NT // 4 - 1),
                        )
                        zp_group = []
                # Z row -> [128, 4] columns, then cheap per-partition 1/Z
                zrow = tp.tile([1, NB], f32, tag="row", bufs=4, name=f"zrow{lb}")
                nc.vector.tensor_copy(zrow, zps)
                zTr = tp.tile([PT, NB // PT], f32, tag="zTr", bufs=2)
                row_to_col(zrow, zTr, f"zT{lb}")
                zT = tp.tile([PT, NB // PT], f32, tag="zT", bufs=2)
                nc.vector.reciprocal(zT, zTr)
                # O~ accumulators -> fp8 pairs (scaled 1/64) for DR out-proj
                obf8 = []
                for mp in range(CCH // 2):
                    ot = tp.tile([PT, 2, NB], f8, tag="obf", bufs=2, name=f"obf{mp}")
                    nc.vector.tensor_scalar_mul(ot[:, 0, :], ops[2 * mp], 1.0 / 64.0)
                    nc.vector.tensor_scalar_mul(ot[:, 1, :], ops[2 * mp + 1], 1.0 / 64.0)
                    obf8.append(ot)
                # final projection; normalize by 1/Z and add bias+residual
                for sub in range(NB // PT):
                    t = lb * (NB // PT) + sub
                    # alternate the two single-buffer PSUM tags (pz is free
                    # once zrow is copied) to double-buffer the epilogue
                    yps = psp.tile(
                        [PT, NB], f32, tag=("py" if sub % 2 == 0 else "pz"),
                        bufs=1, name=f"yps{sub % 2}",
                    )
                    for mp in range(CCH // 2):
                        nc.tensor.matmul(
                            yps,
                            obf8[mp][:, :, sub * PT : (sub + 1) * PT],
                            wb["o"][:, 2 * mp : 2 * mp + 2, :],
                            start=(mp == 0),
                            stop=(mp == CCH // 2 - 1),
                            perf_mode=DR,
                        )
                    xrb = tp.tile([PT, C], f32, tag="xrb", bufs=2)
                    nc.vector.tensor_add(xrb, xr4[:, sub, :], bo_bc)
                    yt = tp.tile([PT, C], f32, tag="yt", bufs=2)
                    nc.vector.scalar_tensor_tensor(
                        out=yt,
                        in0=yps,
                        scalar=zT[:, sub : sub + 1],
                        in1=xrb,
                        op0=mybir.AluOpType.mult,
                        op1=mybir.AluOpType.add,
                    )
                    nc.sync.dma_start(out=y_d[t * PT : (t + 1) * PT, :], in_=yt)

    nc.compile()
    return nc


_NC_CACHE = None


def _get_program():
    global _NC_CACHE
    if _NC_CACHE is None:
        _NC_CACHE = build_program()
    return _NC_CACHE


def make_in_maps(inputs):
    hs = np.ascontiguousarray(np.asarray(inputs["hidden_states"], np.float32))
    ws = {n: np.ascontiguousarray(np.asarray(inputs["w" + n], np.float32)) for n in "qkvo"}
    bs = {n: np.ascontiguousarray(np.asarray(inputs["b" + n], np.float32)) for n in "qkvo"}
    gsc = np.ascontiguousarray(np.asarray(inputs["gn_scale"], np.float32))
    gbi = np.ascontiguousarray(np.asarray(inputs["gn_bias"], np.float32))
    import ml_dtypes
    eye = np.eye(PT, dtype=ml_dtypes.bfloat16)
    eg = np.zeros((G, C), np.float32)
    eg[np.arange(C) // GS, np.arange(C)] = 1.0
    in_maps = []
    for core in range(NCORES):
        b, h = core // 2, core % 2
        xb = hs[b].reshape(L, C)
        x_roll = np.ascontiguousarray(np.roll(xb, -h * LQ, axis=0))
        m = {"x": x_roll, "gn_scale": gsc, "gn_bias": gbi, "egrp": eg, "eye": eye}
        for n in "qkvo":
            m["w" + n] = ws[n]
            m["b" + n] = bs[n]
        in_maps.append(m)
    return in_maps


def assemble(results):
    out = np.empty((B, L, C), np.float32)
    for core in range(NCORES):
        b, h = core // 2, core % 2
        out[b, h * LQ : (h + 1) * LQ] = results[core]["y"]
    return out.reshape(B, HH, WW, C)


def kernel(**inputs):
    from concourse.bass_utils import run_bass_kernel_spmd

    nc = _get_program()
    in_maps = make_in_maps(inputs)
    res = run_bass_kernel_spmd(nc, in_maps, list(range(NCORES)))
    return assemble(res.results)


if __name__ == "__main__":
    rng = np.random.default_rng(0)
    s = 1.0 / np.sqrt(C)
    inputs = {
        "hidden_states": rng.standard_normal((B, HH, WW, C), np.float32),
        "gn_scale": np.ones(C, np.float32),
        "gn_bias": np.zeros(C, np.float32),
    }
    for n in "qkvo":
        inputs["w" + n] = (rng.standard_normal((C, C)) * s).astype(np.float32)
        inputs["b" + n] = np.zeros(C, np.float32)
    out = kernel(**inputs)
    print(out.shape, out.dtype)


# revision 17
# speedup vs baseline: 1.1138x; 1.0997x over previous
"""Trainium2 Bass kernel for GroupNorm + single-head self-attention block.

Reference computation (per batch b):
    xn = GroupNorm(x; 32 groups over (L, C/32)) * gn_scale + gn_bias
    q, k, v = xn@wq+bq, xn@wk+bk, xn@wv+bv
    out = softmax(q k^T / sqrt(C)) v @ wo + bo + x

Sharding: 8 cores = 4 batches x 2 query-halves. Each core receives its
batch's [L=4096, C=512] slice ROTATED so that its 2048 query rows are
always rows 0..2047 (attention and GN stats are invariant to key/value
ordering), which keeps the program SPMD-identical across cores.

On-core dataflow (fp8 DoubleRow matmuls everywhere heavy, fp32 accum):
  - GN is folded into the projections: A[c]=rstd[g]*gn_scale[c],
    B[c]=gn_bias[c]-mean[g]*A[c]; wq'=S*A (.) wq (row scaling),
    b'q = S*(B@wq + bq), with S=16 a power-of-2 pre-scale that keeps
    the fp8 weights out of the subnormal range. wo'=wo*64/S with the
    O~ accumulator scaled by 1/64 in its PSUM->fp8 copy.
  - Phase X: x streams in fp32, is cast to fp8 (xball, SBUF-resident),
    and GN stats accumulate via ones-vector matmuls (fp8 for sum-x,
    bf16 squares from the fp32 stream for sum-x2). The 128 PE
    transposes of xball -> xT [C, L] are emitted AFTER the stats loop
    so they fill the PE while the serial GN-stats chain (DVE/ACT)
    runs; this keeps the HAM clock-gate warm through phases S/WP.
  - Q^T, K^T in fp8 [c, l] layout; V in fp8 [s, c] layout; all
    projection matmuls are fp8 DoubleRow (2 chained 256-deep MMs).
  - Attention is software-pipelined one key-pair ahead: the PE queue
    order is scores(p+1), zsum(p-3), AV(p) so the strict-FIFO engine
    queue never head-of-line blocks on the exp (ACT) of pair p.
    exp uses scale=1/(S^2 sqrt(C)) and bias=-4 (cancels in softmax,
    keeps exp inside fp8 range); exp writes the [128,2,512] DoubleRow
    moving operand for AV directly.
  - Row sums Z come from ones-matmuls over DVE pair-sums of exp tiles,
    deferred 2 pairs so the DVE never gates the PE. 1/Z is taken on a
    [128,4] column layout after a PE transpose of the Z row; the final
    projection output is scaled by 1/Z and fused with bias + residual.
"""

import sys

sys.path.insert(0, "/opt/trn_rl_repo")

import numpy as np

B, HH, WW, C = 4, 64, 64, 512
L = HH * WW          # 4096
G = 32               # groups
GS = C // G          # 16 channels per group
EPS = 1e-6
NCORES = 8
LQ = L // 2          # 2048 query rows per core
PT = 128             # partition tile
NT = L // PT         # 32 row tiles
NTQ = LQ // PT       # 16 query row tiles
CCH = C // PT        # 4 channel chunks
NB = 512             # matmul moving-free block
NP = NT // 2         # 16 key pairs in attention
S = 16.0             # fp8 pre-scale folded into wq/wk/wv rows
SCALE = 1.0 / float(np.sqrt(C))
EXPB = -4.0          # constant shift inside exp (cancels in softmax)


def build_program():
    import concourse.bacc as bacc
    import concourse.bass as bass
    import concourse.mybir as mybir
    import concourse.tile as tile

    f32 = mybir.dt.float32
    bf16 = mybir.dt.bfloat16
    f8 = mybir.dt.float8e4
    AF = mybir.ActivationFunctionType
    DR = mybir.MatmulPerfMode.DoubleRow

    nc = bacc.Bacc(
        trn_type="TRN2",
        target_bir_lowering=False,
        debug=False,
        num_devices=NCORES,
    )

    x_d = nc.dram_tensor("x", [L, C], f32, kind="ExternalInput").ap()
    gs_d = nc.dram_tensor("gn_scale", [C], f32, kind="ExternalInput").ap()
    gb_d = nc.dram_tensor("gn_bias", [C], f32, kind="ExternalInput").ap()
    w_d = {}
    b_d = {}
    for n in "qkvo":
        w_d[n] = nc.dram_tensor("w" + n, [C, C], f32, kind="ExternalInput").ap()
        b_d[n] = nc.dram_tensor("b" + n, [C], f32, kind="ExternalInput").ap()
    eg_d = nc.dram_tensor("egrp", [G, C], f32, kind="ExternalInput").ap()
    eye_d = nc.dram_tensor("eye", [PT, PT], f8, kind="ExternalInput").ap()
    y_d = nc.dram_tensor("y", [LQ, C], f32, kind="ExternalOutput").ap()

    with tile.TileContext(nc) as tc:
        with (
            tc.tile_pool(name="persist", bufs=1) as pp,
            tc.tile_pool(name="trans", bufs=1) as tp,
            tc.tile_pool(name="dram", bufs=1, space="DRAM") as dp,
            tc.tile_pool(name="psum", bufs=1, space="PSUM") as psp,
        ):
            # ---- constants ----
            ones_col = pp.tile([PT, 1], bf16, tag="ones_col")
            nc.vector.memset(ones_col, 1.0)
            ones8 = pp.tile([PT, 1], f8, tag="ones8")
            nc.vector.memset(ones8, 1.0)
            ones_row = pp.tile([1, PT], bf16, tag="ones_row")
            nc.vector.memset(ones_row, 1.0)
            expb_t = pp.tile([PT, 1], f32, tag="expb")
            nc.vector.memset(expb_t, float(EXPB))
            one_f = pp.tile([1, 1], f32, tag="one_f")
            nc.vector.memset(one_f, 1.0)

            # eye is the only DMA ahead of the x stream on the sync ring
            eye_sb = pp.tile([PT, PT], f8, tag="eye")
            nc.sync.dma_start(out=eye_sb, in_=eye_d)
            # group->channel indicator matrix [G, C] (host-side constant)
            eg_sb = pp.tile([G, C], f32, tag="eg")
            nc.gpsimd.dma_start(out=eg_sb, in_=eg_d)

            # ---- persistent tensors ----
            xball = pp.tile([PT, NT, C], f8, tag="xball")       # 2 MB
            xT = pp.tile([PT, CCH, L], f8, tag="xT")            # 2 MB
            qT = pp.tile([PT, CCH, LQ], f8, tag="qT")           # 1 MB
            kT = pp.tile([PT, CCH, L], f8, tag="kT")            # 2 MB
            v_sb = pp.tile([PT, NT, NB], f8, tag="v")           # 2 MB
            wb = {
                n: pp.tile([PT, CCH, C], f8, tag="wb" + n, name="wb_" + n)
                for n in "qkvo"
            }

            def row_to_col(row, out_sb, nm):
                """[1, n*128] row -> [128, n] column layout, via K=1 matmuls."""
                n = out_sb.shape[-1]
                ps = psp.tile(
                    [PT, n], f32, tag="ps", bufs=2, name="r2c_" + nm
                )
                for m_ in range(n):
                    nc.tensor.matmul(
                        ps[:, m_ : m_ + 1],
                        row[0:1, m_ * PT : (m_ + 1) * PT],
                        one_f,
                        start=True,
                        stop=True,
                    )
                nc.vector.tensor_copy(out_sb, ps)

            # ---- small bias/scale loads (gpsimd ring; off the x stream) ----
            bq_row = pp.tile([1, C], f32, tag="bq_row")
            nc.gpsimd.dma_start(out=bq_row, in_=b_d["q"].unsqueeze(0))
            bk_row = pp.tile([1, C], f32, tag="bk_row")
            nc.gpsimd.dma_start(out=bk_row, in_=b_d["k"].unsqueeze(0))
            bv_row = pp.tile([1, C], f32, tag="bv_row")
            nc.gpsimd.dma_start(out=bv_row, in_=b_d["v"].unsqueeze(0))
            bo_bc = pp.tile([PT, C], f32, tag="bo_bc")
            nc.gpsimd.dma_start(
                out=bo_bc, in_=b_d["o"].unsqueeze(0).broadcast_to([PT, C])
            )
            gs_row = pp.tile([1, C], f32, tag="gs_row")
            nc.gpsimd.dma_start(out=gs_row, in_=gs_d.unsqueeze(0))
            gb_row = pp.tile([1, C], f32, tag="gb_row")
            nc.gpsimd.dma_start(out=gb_row, in_=gb_d.unsqueeze(0))

            # ---- phase X: stream x, cast to fp8, GN stats ----
            sum_ps = psp.tile([1, C], f32, tag="pz")
            sq_ps = psp.tile([1, C], f32, tag="py")
            for t in range(NT):
                if t % 2 == 0:
                    g = t // 2
                    xf4 = tp.tile([PT, 2, C], f32, tag="xf4", bufs=3, name=f"xf4_{g}")
                    nc.sync.dma_start(
                        out=xf4,
                        in_=x_d[g * 2 * PT : (g + 1) * 2 * PT, :].rearrange(
                            "(i p) c -> p i c", p=PT
                        ),
                    )
                xf = xf4[:, t % 2, :]
                nc.vector.tensor_copy(xball[:, t, :], xf)
                sq = tp.tile([PT, C], bf16, tag="sq", bufs=2)
                nc.scalar.activation(out=sq, in_=xf, func=AF.Square)
                nc.tensor.matmul(
                    sum_ps, ones8, xball[:, t, :], start=(t == 0), stop=(t == NT - 1)
                )
                nc.tensor.matmul(
                    sq_ps, ones_col, sq, start=(t == 0), stop=(t == NT - 1)
                )

            # ---- weight loads: sync ring AFTER the x stream (FIFO keeps
            # them from competing with x for HBM bandwidth) ----
            wf = {}
            for n in "qkvo":
                wf[n] = tp.tile([PT, CCH, C], f32, tag="wf", bufs=2, name="wf_" + n)
                nc.sync.dma_start(
                    out=wf[n], in_=w_d[n].rearrange("(j p) c -> p j c", p=PT)
                )

            # ---- phase T: transposes, emitted after stats so they fill the
            # PE while the serial stats chain runs on DVE/ACT ----
            for t in range(NT):
                # fp8 transpose-mode requires an output element step of 2,
                # so write into the even lanes of a [.., 2] PSUM view
                t_ps = psp.tile([PT, CCH, PT, 2], f8, tag="po", bufs=4, name=f"tps{t}")
                for j in range(CCH):
                    nc.tensor.transpose(
                        t_ps[:, j, :, 0],
                        xball[:, t, j * PT : (j + 1) * PT],
                        eye_sb,
                    )
                nc.vector.tensor_copy(
                    xT[:, :, t * PT : (t + 1) * PT],
                    t_ps[:, :, :, 0],
                )

            # ---- phase S: GN stats -> A,B rows -> bounce to [128,4] ----
            s1 = tp.tile([1, G], f32, tag="small", bufs=8)
            nc.vector.reduce_sum(
                out=s1,
                in_=sum_ps.rearrange("p (g d) -> p g d", g=G),
                axis=mybir.AxisListType.X,
            )
            s2 = tp.tile([1, G], f32, tag="small", bufs=8)
            nc.vector.reduce_sum(
                out=s2,
                in_=sq_ps.rearrange("p (g d) -> p g d", g=G),
                axis=mybir.AxisListType.X,
            )
            inv_n = 1.0 / float(L * GS)
            mean = tp.tile([1, G], f32, tag="small", bufs=8)
            nc.vector.tensor_scalar_mul(mean, s1, inv_n)
            ex2 = tp.tile([1, G], f32, tag="small", bufs=8)
            nc.vector.tensor_scalar_mul(ex2, s2, inv_n)
            m2 = tp.tile([1, G], f32, tag="small", bufs=8)
            nc.vector.tensor_mul(m2, mean, mean)
            var = tp.tile([1, G], f32, tag="small", bufs=8)
            nc.vector.tensor_sub(var, ex2, m2)
            sd = tp.tile([1, G], f32, tag="small", bufs=8)
            eps_t = tp.tile([1, 1], f32, tag="small", bufs=8)
            nc.vector.memset(eps_t, float(EPS))
            nc.scalar.activation(out=sd, in_=var, func=AF.Sqrt, bias=eps_t)
            rstd = tp.tile([1, G], f32, tag="small", bufs=8)
            nc.vector.reciprocal(rstd, sd)

            # mean/rstd [1,32] -> columns [32,1] -> expand to channel rows
            gcol_ps = psp.tile([G, 2], f32, tag="ps", bufs=2)
            nc.tensor.matmul(gcol_ps[:, 0:1], rstd, one_f, start=True, stop=True)
            nc.tensor.matmul(gcol_ps[:, 1:2], mean, one_f, start=True, stop=True)
            gcol = tp.tile([G, 2], f32, tag="small", bufs=8)
            nc.vector.tensor_copy(gcol, gcol_ps)
            rstd_e_ps = psp.tile([1, C], f32, tag="ps", bufs=2)
            nc.tensor.matmul(rstd_e_ps, gcol[:, 0:1], eg_sb, start=True, stop=True)
            a_row = tp.tile([1, C], f32, tag="row", bufs=4)
            nc.vector.tensor_mul(a_row, rstd_e_ps, gs_row)
            mean_e_ps = psp.tile([1, C], f32, tag="ps", bufs=2)
            nc.tensor.matmul(mean_e_ps, gcol[:, 1:2], eg_sb, start=True, stop=True)
            mb = tp.tile([1, C], f32, tag="row", bufs=4)
            nc.vector.tensor_mul(mb, mean_e_ps, a_row)
            b_row = tp.tile([1, C], f32, tag="row", bufs=4)
            nc.vector.tensor_sub(b_row, gb_row, mb)
            # S-scaled A column (folded into wq/wk/wv rows)
            a16_row = tp.tile([1, C], f32, tag="row", bufs=4)
            nc.vector.tensor_scalar_mul(a16_row, a_row, float(S))
            aT = pp.tile([PT, CCH], f32, tag="aT")
            row_to_col(a16_row, aT, "aT")
            bT = pp.tile([PT, CCH], f32, tag="bT")
            row_to_col(b_row, bT, "bT")
            bT_bf = pp.tile([PT, CCH], bf16, tag="bT_bf")
            nc.vector.tensor_copy(bT_bf, bT)

            # bf16 copies of wq/wk/wv for the (tiny) B@w bias-fold matmuls —
            # bf16 matmuls run 4x faster than fp32 ones
            wfb = {}
            for n in "qkv":
                wfb[n] = tp.tile([PT, CCH, C], bf16, tag="wfb", bufs=2, name="wfb_" + n)
                for j in range(CCH):
                    nc.vector.tensor_copy(wfb[n][:, j, :], wf[n][:, j, :])

            # ---- phase WP: fold GN into weights & biases ----
            # b'q/b'k = S*(B @ w + b), computed as rows then moved to columns
            bq_f = pp.tile([PT, CCH], f32, tag="bq_f")
            bk_f = pp.tile([PT, CCH], f32, tag="bk_f")
            for n, bias_row, out_t in (("q", bq_row, bq_f), ("k", bk_row, bk_f)):
                psb = psp.tile([1, C], f32, tag="ps", bufs=2, name="psb_" + n)
                for j in range(CCH):
                    nc.tensor.matmul(
                        psb,
                        bT_bf[:, j : j + 1],
                        wfb[n][:, j, :],
                        start=(j == 0),
                        stop=(j == CCH - 1),
                    )
                bp_row = tp.tile([1, C], f32, tag="row", bufs=4, name="bp_" + n)
                nc.vector.tensor_add(bp_row, psb, bias_row)
                bp16_row = tp.tile([1, C], f32, tag="row", bufs=4, name="bp16_" + n)
                nc.vector.tensor_scalar_mul(bp16_row, bp_row, float(S))
                row_to_col(bp16_row, out_t, "b" + n)
            # b'v as a row [1, 512] (bias enters V via ones-row matmul)
            psv = psp.tile([1, C], f32, tag="pz")
            for j in range(CCH):
                nc.tensor.matmul(
                    psv,
                    bT_bf[:, j : j + 1],
                    wfb["v"][:, j, :],
                    start=(j == 0),
                    stop=(j == CCH - 1),
                )
            bvp = tp.tile([1, C], f32, tag="row", bufs=4)
            nc.vector.tensor_add(bvp, psv, bv_row)
            bvp16 = tp.tile([1, C], f32, tag="row", bufs=4)
            nc.vector.tensor_scalar_mul(bvp16, bvp, float(S))
            bvp_bf = pp.tile([1, C], bf16, tag="bvp_bf")
            nc.vector.tensor_copy(bvp_bf, bvp16)
            # b'v broadcast across partitions (for DVE bias add in V copies)
            bvbc_ps = psp.tile([PT, C], f32, tag="ps", bufs=2)
            nc.tensor.matmul(bvbc_ps, ones_row, bvp_bf, start=True, stop=True)
            bvbc = pp.tile([PT, C], f32, tag="bvbc")
            nc.vector.tensor_copy(bvbc, bvbc_ps)

            # scale+cast weights: wq/k/v rows scaled by S*A (per input channel)
            for n in "qkv":
                for j in range(CCH):
                    nc.vector.tensor_scalar_mul(
                        wb[n][:, j, :], wf[n][:, j, :], aT[:, j : j + 1]
                    )
            # wo in fp8 scaled by 64/S: the O~ accumulator is scaled by 1/64
            # in the PSUM->fp8 copy, so the product keeps the same scale
            for j in range(CCH):
                nc.vector.tensor_scalar_mul(
                    wb["o"][:, j, :], wf["o"][:, j, :], 64.0 / float(S)
                )

            # ---- phase P: projections (fp8 DoubleRow, 2x256-deep chains) ----
            # Q^T [c, l] for query rows 0..2047
            for m in range(CCH):
                for lb in range(LQ // NB):
                    ps = psp.tile([PT, NB], f32, tag="po", bufs=4)
                    for jp in range(CCH // 2):
                        nc.tensor.matmul(
                            ps,
                            wb["q"][:, 2 * jp : 2 * jp + 2, m * PT : (m + 1) * PT],
                            xT[:, 2 * jp : 2 * jp + 2, lb * NB : (lb + 1) * NB],
                            start=(jp == 0),
                            stop=(jp == CCH // 2 - 1),
                            perf_mode=DR,
                        )
                    nc.vector.tensor_scalar_add(
                        qT[:, m, lb * NB : (lb + 1) * NB], ps, bq_f[:, m : m + 1]
                    )
            # K^T [c, l] for all rows
            for m in range(CCH):
                for lb in range(L // NB):
                    ps = psp.tile([PT, NB], f32, tag="po", bufs=4)
                    for jp in range(CCH // 2):
                        nc.tensor.matmul(
                            ps,
                            wb["k"][:, 2 * jp : 2 * jp + 2, m * PT : (m + 1) * PT],
                            xT[:, 2 * jp : 2 * jp + 2, lb * NB : (lb + 1) * NB],
                            start=(jp == 0),
                            stop=(jp == CCH // 2 - 1),
                            perf_mode=DR,
                        )
                    nc.vector.tensor_scalar_add(
                        kT[:, m, lb * NB : (lb + 1) * NB], ps, bk_f[:, m : m + 1]
                    )
            # V natural [s, c] for all rows; bias via broadcast add
            for t in range(NT):
                ps = psp.tile([PT, NB], f32, tag="po", bufs=4)
                for jp in range(CCH // 2):
                    nc.tensor.matmul(
                        ps,
                        xT[:, 2 * jp : 2 * jp + 2, t * PT : (t + 1) * PT],
                        wb["v"][:, 2 * jp : 2 * jp + 2, :],
                        start=(jp == 0),
                        stop=(jp == CCH // 2 - 1),
                        perf_mode=DR,
                    )
                nc.vector.tensor_add(v_sb[:, t, :], ps, bvbc)

            # ---- phase A: attention, one 512-wide query block at a time,
            # software-pipelined one key-pair ahead ----
            for lb in range(LQ // NB):
                zps = psp.tile([1, NB], f32, tag="pz")
                ops = [
                    psp.tile([PT, NB], f32, tag="po", bufs=4, name=f"ops{m}")
                    for m in range(CCH)
                ]
                # prefetch the residual rows for this block's epilogue
                xr4 = tp.tile([PT, 4, C], f32, tag="xr4", bufs=2, name=f"xr4_{lb}")
                nc.sync.dma_start(
                    out=xr4,
                    in_=x_d[lb * NB : (lb + 1) * NB, :].rearrange(
                        "(i p) c -> p i c", p=PT
                    ),
                )

                a_pairs = {}
                zp_t = {}
                z4_t = {}

                def emit_scores(p):
                    a_pair = tp.tile([PT, 2, NB], f8, tag="a_t", bufs=3)
                    a_pairs[p] = a_pair
                    for half in range(2):
                        st = 2 * p + half
                        sps = psp.tile([PT, NB], f32, tag="ps", bufs=2)
                        for jp in range(CCH // 2):
                            nc.tensor.matmul(
                                sps,
                                kT[:, 2 * jp : 2 * jp + 2, st * PT : (st + 1) * PT],
                                qT[:, 2 * jp : 2 * jp + 2, lb * NB : (lb + 1) * NB],
                                start=(jp == 0),
                                stop=(jp == CCH // 2 - 1),
                                perf_mode=DR,
                            )
                        nc.scalar.activation(
                            out=a_pair[:, half, :],
                            in_=sps,
                            func=AF.Exp,
                            scale=SCALE / float(S * S),
                            bias=expb_t,
                        )

                def emit_zmm(g):
                    nc.tensor.matmul(
                        zps,
                        ones_col,
                        z4_t.pop(g),
                        start=(g == 0),
                        stop=(g == NP // 2 - 1),
                    )

                emit_scores(0)
                for p in range(NP):
                    if p + 1 < NP:
                        emit_scores(p + 1)
                    # Z matmul for pair-group (p-3)//2 — deferred so the DVE
                    # pair-sums are long done and never stall the PE queue
                    if p >= 3 and p % 2 == 1:
                        emit_zmm((p - 3) // 2)
                    a_pair = a_pairs.pop(p)
                    for m in range(CCH):
                        nc.tensor.matmul(
                            ops[m],
                            v_sb[:, 2 * p : 2 * p + 2, m * PT : (m + 1) * PT],
                            a_pair,
                            start=(p == 0),
                            stop=(p == NP - 1),
                            perf_mode=DR,
                        )
                    # DVE pair-sum for Z
                    zp = tp.tile([PT, NB], bf16, tag="zp", bufs=3, name=f"zp{p % 3}")
                    nc.vector.tensor_add(zp, a_pair[:, 0, :], a_pair[:, 1, :])
                    zp_t[p] = zp
                    if p % 2 == 1:
                        g = p // 2
                        z4 = tp.tile([PT, NB], bf16, tag="z4", bufs=3)
                        nc.vector.tensor_add(z4, zp_t.pop(p - 1), zp_t.pop(p))
                        z4_t[g] = z4
                emit_zmm(NP // 2 - 1)

                # Z row -> [128, 4] columns, then cheap per-partition 1/Z
                zrow = tp.tile([1, NB], f32, tag="row", bufs=4, name=f"zrow{lb}")
                nc.vector.tensor_copy(zrow, zps)
                zTr = tp.tile([PT, NB // PT], f32, tag="zTr", bufs=2)
                row_to_col(zrow, zTr, f"zT{lb}")
                zT = tp.tile([PT, NB // PT], f32, tag="zT", bufs=2)
                nc.vector.reciprocal(zT, zTr)
                # O~ accumulators -> fp8 pairs (scaled 1/64) for DR out-proj
                obf8 = []
                for mp in range(CCH // 2):
                    ot = tp.tile([PT, 2, NB], f8, tag="obf", bufs=2, name=f"obf{mp}")
                    nc.vector.tensor_scalar_mul(ot[:, 0, :], ops[2 * mp], 1.0 / 64.0)
                    nc.vector.tensor_scalar_mul(ot[:, 1, :], ops[2 * mp + 1], 1.0 / 64.0)
                    obf8.append(ot)
                # final projection; normalize by 1/Z and add bias+residual
                for sub in range(NB // PT):
                    t = lb * (NB // PT) + sub
                    # alternate the two single-buffer PSUM tags (pz is free
                    # once zrow is copied) to double-buffer the epilogue
                    yps = psp.tile(
                        [PT, NB], f32, tag=("py" if sub % 2 == 0 else "pz"),
                        bufs=1, name=f"yps{sub % 2}",
                    )
                    for mp in range(CCH // 2):
                        nc.tensor.matmul(
                            yps,
                            obf8[mp][:, :, sub * PT : (sub + 1) * PT],
                            wb["o"][:, 2 * mp : 2 * mp + 2, :],
                            start=(mp == 0),
                            stop=(mp == CCH // 2 - 1),
                            perf_mode=DR,
                        )
                    xrb = tp.tile([PT, C], f32, tag="xrb", bufs=2)
                    nc.vector.tensor_add(xrb, xr4[:, sub, :], bo_bc)
                    yt = tp.tile([PT, C], f32, tag="yt", bufs=2)
                    nc.vector.scalar_tensor_tensor(
                        out=yt,
                        in0=yps,
                        scalar=zT[:, sub : sub + 1],
                        in1=xrb,
                        op0=mybir.AluOpType.mult,
                        op1=mybir.AluOpType.add,
                    )
                    nc.sync.dma_start(out=y_d[t * PT : (t + 1) * PT, :], in_=yt)

    nc.compile()
    return nc


_NC_CACHE = None


def _get_program():
    global _NC_CACHE
    if _NC_CACHE is None:
        _NC_CACHE = build_program()
    return _NC_CACHE


def make_in_maps(inputs):
    hs = np.ascontiguousarray(np.asarray(inputs["hidden_states"], np.float32))
    ws = {n: np.ascontiguousarray(np.asarray(inputs["w" + n], np.float32)) for n in "qkvo"}
    bs = {n: np.ascontiguousarray(np.asarray(inputs["b" + n], np.float32)) for n in "qkvo"}
    gsc = np.ascontiguousarray(np.asarray(inputs["gn_scale"], np.float32))
    gbi = np.ascontiguousarray(np.asarray(inputs["gn_bias"], np.float32))
    import ml_dtypes
    eye = np.eye(PT, dtype=ml_dtypes.float8_e4m3)
    eg = np.zeros((G, C), np.float32)
    eg[np.arange(C) // GS, np.arange(C)] = 1.0
    in_maps = []
    for core in range(NCORES):
        b, h = core // 2, core % 2
        xb = hs[b].reshape(L, C)
        x_roll = np.ascontiguousarray(np.roll(xb, -h * LQ, axis=0))
        m = {"x": x_roll, "gn_scale": gsc, "gn_bias": gbi, "egrp": eg, "eye": eye}
        for n in "qkvo":
            m["w" + n] = ws[n]
            m["b" + n] = bs[n]
        in_maps.append(m)
    return in_maps


def assemble(results):
    out = np.empty((B, L, C), np.float32)
    for core in range(NCORES):
        b, h = core // 2, core % 2
        out[b, h * LQ : (h + 1) * LQ] = results[core]["y"]
    return out.reshape(B, HH, WW, C)


def kernel(**inputs):
    from concourse.bass_utils import run_bass_kernel_spmd

    nc = _get_program()
    in_maps = make_in_maps(inputs)
    res = run_bass_kernel_spmd(nc, in_maps, list(range(NCORES)))
    return assemble(res.results)


if __name__ == "__main__":
    rng = np.random.default_rng(0)
    s = 1.0 / np.sqrt(C)
    inputs = {
        "hidden_states": rng.standard_normal((B, HH, WW, C), np.float32),
        "gn_scale": np.ones(C, np.float32),
        "gn_bias": np.zeros(C, np.float32),
    }
    for n in "qkvo":
        inputs["w" + n] = (rng.standard_normal((C, C)) * s).astype(np.float32)
        inputs["b" + n] = np.zeros(C, np.float32)
    out = kernel(**inputs)
    print(out.shape, out.dtype)


# revision 18
# speedup vs baseline: 1.2131x; 1.0891x over previous
"""Trainium2 Bass kernel for GroupNorm + single-head self-attention block.

Reference computation (per batch b):
    xn = GroupNorm(x; 32 groups over (L, C/32)) * gn_scale + gn_bias
    q, k, v = xn@wq+bq, xn@wk+bk, xn@wv+bv
    out = softmax(q k^T / sqrt(C)) v @ wo + bo + x

Sharding: 8 cores = 4 batches x 2 query-halves. Each core receives its
batch's [L=4096, C=512] slice ROTATED so that its 2048 query rows are
always rows 0..2047 (attention and GN stats are invariant to key/value
ordering), which keeps the program SPMD-identical across cores.

On-core dataflow (fp8 DoubleRow matmuls everywhere heavy, fp32 accum):
  - x is pre-cast to bf16 on the host (stats, transposes and the
    residual all tolerate bf16), halving the input stream to 4MB.
  - GN is folded into the projections: A[c]=rstd[g]*gn_scale[c],
    B[c]=gn_bias[c]-mean[g]*A[c]; wq'=S*A (.) wq (row scaling),
    b'q = S*(B@wq + bq), with S=16 a power-of-2 pre-scale that keeps
    the fp8 weights out of the subnormal range. wo'=wo*64/S with the
    O~ accumulator scaled by 1/64 in its PSUM->fp8 copy.
  - Phase X: x streams in bf16, is cast to fp8 (xball, SBUF-resident);
    GN stats accumulate via fp8 DoubleRow ones-matmuls (x and fp8
    squares from ACT). The 128 PE transposes of xball -> xT [C, L] are
    emitted AFTER the stats loop so they fill the PE while the serial
    GN-stats chain (DVE/ACT) runs, keeping the HAM clock-gate warm.
  - Q^T/K^T PSUM tiles drain through ACT (Identity + per-partition
    bias column) because DVE throughput, not PE, limits the projection
    phase; V drains stay on DVE (its bias varies along the free dim).
  - Attention is software-pipelined one key-pair ahead ACROSS query
    blocks: the PE queue order is scores(next pair), zsum(deferred),
    AV(current), with each block's epilogue emitted after the next
    block's first scores, so the strict-FIFO engine queue never
    head-of-line blocks on ACT exps or DVE drains.
  - Row sums Z come from ones-matmuls over DVE pair-sums of exp tiles,
    deferred 2 pairs; 1/Z is taken on a [128,4] column layout after a
    PE transpose of the Z row; the final DR projection is scaled by
    1/Z and fused with bias + residual in one DVE op.
"""

import sys

sys.path.insert(0, "/opt/trn_rl_repo")

import numpy as np

B, HH, WW, C = 4, 64, 64, 512
L = HH * WW          # 4096
G = 32               # groups
GS = C // G          # 16 channels per group
EPS = 1e-6
NCORES = 8
LQ = L // 2          # 2048 query rows per core
PT = 128             # partition tile
NT = L // PT         # 32 row tiles
CCH = C // PT        # 4 channel chunks
NB = 512             # matmul moving-free block
NP = NT // 2         # 16 key pairs in attention
NLB = LQ // NB       # 4 query blocks
S = 16.0             # fp8 pre-scale folded into wq/wk/wv rows
SCALE = 1.0 / float(np.sqrt(C))
EXPB = -4.0          # constant shift inside exp (cancels in softmax)


def build_program():
    import concourse.bacc as bacc
    import concourse.bass as bass
    import concourse.mybir as mybir
    import concourse.tile as tile

    f32 = mybir.dt.float32
    bf16 = mybir.dt.bfloat16
    f8 = mybir.dt.float8e4
    AF = mybir.ActivationFunctionType
    DR = mybir.MatmulPerfMode.DoubleRow

    nc = bacc.Bacc(
        trn_type="TRN2",
        target_bir_lowering=False,
        debug=False,
        num_devices=NCORES,
    )

    x_d = nc.dram_tensor("x", [L, C], bf16, kind="ExternalInput").ap()
    gs_d = nc.dram_tensor("gn_scale", [C], f32, kind="ExternalInput").ap()
    gb_d = nc.dram_tensor("gn_bias", [C], f32, kind="ExternalInput").ap()
    w_d = {}
    b_d = {}
    for n in "qkvo":
        w_d[n] = nc.dram_tensor("w" + n, [C, C], f32, kind="ExternalInput").ap()
        b_d[n] = nc.dram_tensor("b" + n, [C], f32, kind="ExternalInput").ap()
    eg_d = nc.dram_tensor("egrp", [G, C], f32, kind="ExternalInput").ap()
    eye_d = nc.dram_tensor("eye", [PT, PT], f8, kind="ExternalInput").ap()
    y_d = nc.dram_tensor("y", [LQ, C], f32, kind="ExternalOutput").ap()

    with tile.TileContext(nc) as tc:
        with (
            tc.tile_pool(name="persist", bufs=1) as pp,
            tc.tile_pool(name="trans", bufs=1) as tp,
            tc.tile_pool(name="dram", bufs=1, space="DRAM") as dp,
            tc.tile_pool(name="psum", bufs=1, space="PSUM") as psp,
        ):
            # ---- constants ----
            ones_col = pp.tile([PT, 1], bf16, tag="ones_col")
            nc.vector.memset(ones_col, 1.0)
            # fp8 DoubleRow ones for the stats matmuls; middle-dim byte
            # step must be 16-aligned, hence the padded [PT, 2, 16] tile
            ones82 = pp.tile([PT, 2, 16], f8, tag="ones82")
            nc.vector.memset(ones82, 1.0)
            ones_row = pp.tile([1, PT], bf16, tag="ones_row")
            nc.vector.memset(ones_row, 1.0)
            expb_t = pp.tile([PT, 1], f32, tag="expb")
            nc.vector.memset(expb_t, float(EXPB))
            one_f = pp.tile([1, 1], f32, tag="one_f")
            nc.vector.memset(one_f, 1.0)

            # eye is the only DMA ahead of the x stream on the sync ring
            eye_sb = pp.tile([PT, PT], f8, tag="eye")
            nc.sync.dma_start(out=eye_sb, in_=eye_d)
            # group->channel indicator matrix [G, C] (host-side constant)
            eg_sb = pp.tile([G, C], f32, tag="eg")
            nc.gpsimd.dma_start(out=eg_sb, in_=eg_d)

            # ---- persistent tensors ----
            xball = pp.tile([PT, NT, C], f8, tag="xball")       # 2 MB
            xT = pp.tile([PT, CCH, L], f8, tag="xT")            # 2 MB
            qT = pp.tile([PT, CCH, LQ], f8, tag="qT")           # 1 MB
            kT = pp.tile([PT, CCH, L], f8, tag="kT")            # 2 MB
            v_sb = pp.tile([PT, NT, NB], f8, tag="v")           # 2 MB
            wb = {
                n: pp.tile([PT, CCH, C], f8, tag="wb" + n, name="wb_" + n)
                for n in "qkvo"
            }

            def row_to_col(row, out_sb, nm):
                """[1, n*128] row -> [128, n] column layout, via K=1 matmuls."""
                n = out_sb.shape[-1]
                ps = psp.tile(
                    [PT, n], f32, tag="ps", bufs=2, name="r2c_" + nm
                )
                for m_ in range(n):
                    nc.tensor.matmul(
                        ps[:, m_ : m_ + 1],
                        row[0:1, m_ * PT : (m_ + 1) * PT],
                        one_f,
                        start=True,
                        stop=True,
                    )
                nc.vector.tensor_copy(out_sb, ps)

            # ---- small bias/scale loads (gpsimd ring; off the x stream) ----
            bq_row = pp.tile([1, C], f32, tag="bq_row")
            nc.gpsimd.dma_start(out=bq_row, in_=b_d["q"].unsqueeze(0))
            bk_row = pp.tile([1, C], f32, tag="bk_row")
            nc.gpsimd.dma_start(out=bk_row, in_=b_d["k"].unsqueeze(0))
            bv_row = pp.tile([1, C], f32, tag="bv_row")
            nc.gpsimd.dma_start(out=bv_row, in_=b_d["v"].unsqueeze(0))
            bo_bc = pp.tile([PT, C], f32, tag="bo_bc")
            nc.gpsimd.dma_start(
                out=bo_bc, in_=b_d["o"].unsqueeze(0).broadcast_to([PT, C])
            )
            gs_row = pp.tile([1, C], f32, tag="gs_row")
            nc.gpsimd.dma_start(out=gs_row, in_=gs_d.unsqueeze(0))
            gb_row = pp.tile([1, C], f32, tag="gb_row")
            nc.gpsimd.dma_start(out=gb_row, in_=gb_d.unsqueeze(0))

            # ---- phase X: stream x (bf16), cast to fp8, GN stats ----
            sum_ps = psp.tile([1, C], f32, tag="pz")
            sq_ps = psp.tile([1, C], f32, tag="py")
            for t2 in range(NT // 2):
                xf4 = tp.tile([PT, 2, C], bf16, tag="xf4", bufs=3, name=f"xf4_{t2}")
                nc.sync.dma_start(
                    out=xf4,
                    in_=x_d[t2 * 2 * PT : (t2 + 1) * 2 * PT, :].rearrange(
                        "(i p) c -> p i c", p=PT
                    ),
                )
                sq2 = tp.tile([PT, 2, C], f8, tag="sq", bufs=2)
                for i in range(2):
                    nc.vector.tensor_copy(xball[:, 2 * t2 + i, :], xf4[:, i, :])
                    nc.scalar.activation(
                        out=sq2[:, i, :], in_=xf4[:, i, :], func=AF.Square
                    )
                nc.tensor.matmul(
                    sum_ps,
                    ones82[:, :, 0:1],
                    xball[:, 2 * t2 : 2 * t2 + 2, :],
                    start=(t2 == 0),
                    stop=(t2 == NT // 2 - 1),
                    perf_mode=DR,
                )
                nc.tensor.matmul(
                    sq_ps,
                    ones82[:, :, 0:1],
                    sq2,
                    start=(t2 == 0),
                    stop=(t2 == NT // 2 - 1),
                    perf_mode=DR,
                )

            # ---- weight loads: sync ring AFTER the x stream (FIFO keeps
            # them from competing with x for HBM bandwidth) ----
            wf = {}
            for n in "qkvo":
                wf[n] = tp.tile([PT, CCH, C], f32, tag="wf", bufs=2, name="wf_" + n)
                nc.sync.dma_start(
                    out=wf[n], in_=w_d[n].rearrange("(j p) c -> p j c", p=PT)
                )

            # ---- phase T: transposes, emitted after stats so they fill the
            # PE while the serial stats chain runs on DVE/ACT ----
            for t in range(NT):
                # fp8 transpose-mode requires an output element step of 2,
                # so write into the even lanes of a [.., 2] PSUM view
                t_ps = psp.tile([PT, CCH, PT, 2], f8, tag="po", bufs=4, name=f"tps{t}")
                for j in range(CCH):
                    nc.tensor.transpose(
                        t_ps[:, j, :, 0],
                        xball[:, t, j * PT : (j + 1) * PT],
                        eye_sb,
                    )
                nc.vector.tensor_copy(
                    xT[:, :, t * PT : (t + 1) * PT],
                    t_ps[:, :, :, 0],
                )

            # ---- phase S: GN stats -> A,B rows -> bounce to [128,4] ----
            s1 = tp.tile([1, G], f32, tag="small", bufs=8)
            nc.vector.reduce_sum(
                out=s1,
                in_=sum_ps.rearrange("p (g d) -> p g d", g=G),
                axis=mybir.AxisListType.X,
            )
            s2 = tp.tile([1, G], f32, tag="small", bufs=8)
            nc.vector.reduce_sum(
                out=s2,
                in_=sq_ps.rearrange("p (g d) -> p g d", g=G),
                axis=mybir.AxisListType.X,
            )
            inv_n = 1.0 / float(L * GS)
            mean = tp.tile([1, G], f32, tag="small", bufs=8)
            nc.vector.tensor_scalar_mul(mean, s1, inv_n)
            ex2 = tp.tile([1, G], f32, tag="small", bufs=8)
            nc.vector.tensor_scalar_mul(ex2, s2, inv_n)
            m2 = tp.tile([1, G], f32, tag="small", bufs=8)
            nc.vector.tensor_mul(m2, mean, mean)
            var = tp.tile([1, G], f32, tag="small", bufs=8)
            nc.vector.tensor_sub(var, ex2, m2)
            sd = tp.tile([1, G], f32, tag="small", bufs=8)
            eps_t = tp.tile([1, 1], f32, tag="small", bufs=8)
            nc.vector.memset(eps_t, float(EPS))
            nc.scalar.activation(out=sd, in_=var, func=AF.Sqrt, bias=eps_t)
            rstd = tp.tile([1, G], f32, tag="small", bufs=8)
            nc.vector.reciprocal(rstd, sd)

            # mean/rstd [1,32] -> columns [32,1] -> expand to channel rows
            gcol_ps = psp.tile([G, 2], f32, tag="ps", bufs=2)
            nc.tensor.matmul(gcol_ps[:, 0:1], rstd, one_f, start=True, stop=True)
            nc.tensor.matmul(gcol_ps[:, 1:2], mean, one_f, start=True, stop=True)
            gcol = tp.tile([G, 2], f32, tag="small", bufs=8)
            nc.vector.tensor_copy(gcol, gcol_ps)
            rstd_e_ps = psp.tile([1, C], f32, tag="ps", bufs=2)
            nc.tensor.matmul(rstd_e_ps, gcol[:, 0:1], eg_sb, start=True, stop=True)
            a_row = tp.tile([1, C], f32, tag="row", bufs=4)
            nc.vector.tensor_mul(a_row, rstd_e_ps, gs_row)
            mean_e_ps = psp.tile([1, C], f32, tag="ps", bufs=2)
            nc.tensor.matmul(mean_e_ps, gcol[:, 1:2], eg_sb, start=True, stop=True)
            mb = tp.tile([1, C], f32, tag="row", bufs=4)
            nc.vector.tensor_mul(mb, mean_e_ps, a_row)
            b_row = tp.tile([1, C], f32, tag="row", bufs=4)
            nc.vector.tensor_sub(b_row, gb_row, mb)
            # S-scaled A column (folded into wq/wk/wv rows)
            a16_row = tp.tile([1, C], f32, tag="row", bufs=4)
            nc.vector.tensor_scalar_mul(a16_row, a_row, float(S))
            aT = pp.tile([PT, CCH], f32, tag="aT")
            row_to_col(a16_row, aT, "aT")
            bT = pp.tile([PT, CCH], f32, tag="bT")
            row_to_col(b_row, bT, "bT")
            bT_bf = pp.tile([PT, CCH], bf16, tag="bT_bf")
            nc.vector.tensor_copy(bT_bf, bT)

            # bf16 copies of wq/wk/wv for the (tiny) B@w bias-fold matmuls —
            # bf16 matmuls run 4x faster than fp32 ones
            wfb = {}
            for n in "qkv":
                wfb[n] = tp.tile([PT, CCH, C], bf16, tag="wfb", bufs=2, name="wfb_" + n)
                for j in range(CCH):
                    nc.vector.tensor_copy(wfb[n][:, j, :], wf[n][:, j, :])

            # ---- phase WP: fold GN into weights & biases ----
            # b'q/b'k = S*(B @ w + b), computed as rows then moved to columns
            bq_f = pp.tile([PT, CCH], f32, tag="bq_f")
            bk_f = pp.tile([PT, CCH], f32, tag="bk_f")
            for n, bias_row, out_t in (("q", bq_row, bq_f), ("k", bk_row, bk_f)):
                psb = psp.tile([1, C], f32, tag="ps", bufs=2, name="psb_" + n)
                for j in range(CCH):
                    nc.tensor.matmul(
                        psb,
                        bT_bf[:, j : j + 1],
                        wfb[n][:, j, :],
                        start=(j == 0),
                        stop=(j == CCH - 1),
                    )
                bp_row = tp.tile([1, C], f32, tag="row", bufs=4, name="bp_" + n)
                nc.vector.tensor_add(bp_row, psb, bias_row)
                bp16_row = tp.tile([1, C], f32, tag="row", bufs=4, name="bp16_" + n)
                nc.vector.tensor_scalar_mul(bp16_row, bp_row, float(S))
                row_to_col(bp16_row, out_t, "b" + n)
            # b'v as a row [1, 512] (bias enters V via ones-row matmul)
            psv = psp.tile([1, C], f32, tag="pz")
            for j in range(CCH):
                nc.tensor.matmul(
                    psv,
                    bT_bf[:, j : j + 1],
                    wfb["v"][:, j, :],
                    start=(j == 0),
                    stop=(j == CCH - 1),
                )
            bvp = tp.tile([1, C], f32, tag="row", bufs=4)
            nc.vector.tensor_add(bvp, psv, bv_row)
            bvp16 = tp.tile([1, C], f32, tag="row", bufs=4)
            nc.vector.tensor_scalar_mul(bvp16, bvp, float(S))
            bvp_bf = pp.tile([1, C], bf16, tag="bvp_bf")
            nc.vector.tensor_copy(bvp_bf, bvp16)
            # b'v broadcast across partitions (for DVE bias add in V copies)
            bvbc_ps = psp.tile([PT, C], f32, tag="ps", bufs=2)
            nc.tensor.matmul(bvbc_ps, ones_row, bvp_bf, start=True, stop=True)
            bvbc = pp.tile([PT, C], f32, tag="bvbc")
            nc.vector.tensor_copy(bvbc, bvbc_ps)

            # scale+cast weights: wq/k/v rows scaled by S*A (per input channel)
            for n in "qkv":
                for j in range(CCH):
                    nc.vector.tensor_scalar_mul(
                        wb[n][:, j, :], wf[n][:, j, :], aT[:, j : j + 1]
                    )
            # wo in fp8 scaled by 64/S: the O~ accumulator is scaled by 1/64
            # in the PSUM->fp8 copy, so the product keeps the same scale
            for j in range(CCH):
                nc.vector.tensor_scalar_mul(
                    wb["o"][:, j, :], wf["o"][:, j, :], 64.0 / float(S)
                )

            # ---- phase P: projections (fp8 DoubleRow, 2x256-deep chains).
            # Q^T/K^T PSUM tiles drain through ACT (Identity + bias column);
            # V drains through DVE (bias varies along the free dim). ----
            for m in range(CCH):
                for lb in range(LQ // NB):
                    ps = psp.tile([PT, NB], f32, tag="po", bufs=4)
                    for jp in range(CCH // 2):
                        nc.tensor.matmul(
                            ps,
                            wb["q"][:, 2 * jp : 2 * jp + 2, m * PT : (m + 1) * PT],
                            xT[:, 2 * jp : 2 * jp + 2, lb * NB : (lb + 1) * NB],
                            start=(jp == 0),
                            stop=(jp == CCH // 2 - 1),
                            perf_mode=DR,
                        )
                    nc.scalar.activation(
                        out=qT[:, m, lb * NB : (lb + 1) * NB],
                        in_=ps,
                        func=AF.Identity,
                        bias=bq_f[:, m : m + 1],
                    )
            for m in range(CCH):
                for lb in range(L // NB):
                    ps = psp.tile([PT, NB], f32, tag="po", bufs=4)
                    for jp in range(CCH // 2):
                        nc.tensor.matmul(
                            ps,
                            wb["k"][:, 2 * jp : 2 * jp + 2, m * PT : (m + 1) * PT],
                            xT[:, 2 * jp : 2 * jp + 2, lb * NB : (lb + 1) * NB],
                            start=(jp == 0),
                            stop=(jp == CCH // 2 - 1),
                            perf_mode=DR,
                        )
                    nc.scalar.activation(
                        out=kT[:, m, lb * NB : (lb + 1) * NB],
                        in_=ps,
                        func=AF.Identity,
                        bias=bk_f[:, m : m + 1],
                    )
            # V natural [s, c] for all rows; bias via broadcast add on DVE
            for t in range(NT):
                ps = psp.tile([PT, NB], f32, tag="po", bufs=4)
                for jp in range(CCH // 2):
                    nc.tensor.matmul(
                        ps,
                        xT[:, 2 * jp : 2 * jp + 2, t * PT : (t + 1) * PT],
                        wb["v"][:, 2 * jp : 2 * jp + 2, :],
                        start=(jp == 0),
                        stop=(jp == CCH // 2 - 1),
                        perf_mode=DR,
                    )
                nc.vector.tensor_add(v_sb[:, t, :], ps, bvbc)

            # ---- phase A: attention, software-pipelined one key-pair ahead
            # across the four 512-wide query blocks ----
            ctx = {}

            def ensure_ctx(lb):
                if lb in ctx:
                    return ctx[lb]
                zps = psp.tile([1, NB], f32, tag="pz", name=f"zps{lb}")
                ops = [
                    psp.tile([PT, NB], f32, tag="po", bufs=4, name=f"ops{m}")
                    for m in range(CCH)
                ]
                xr4 = tp.tile([PT, 4, C], bf16, tag="xr4", bufs=2, name=f"xr4_{lb}")
                nc.sync.dma_start(
                    out=xr4,
                    in_=x_d[lb * NB : (lb + 1) * NB, :].rearrange(
                        "(i p) c -> p i c", p=PT
                    ),
                )
                ctx[lb] = dict(zps=zps, ops=ops, xr4=xr4, a={}, zp={}, z4={})
                return ctx[lb]

            def emit_scores(lb, p):
                c = ensure_ctx(lb)
                a_pair = tp.tile([PT, 2, NB], f8, tag="a_t", bufs=3)
                c["a"][p] = a_pair
                for half in range(2):
                    st = 2 * p + half
                    sps = psp.tile([PT, NB], f32, tag="ps", bufs=2)
                    for jp in range(CCH // 2):
                        nc.tensor.matmul(
                            sps,
                            kT[:, 2 * jp : 2 * jp + 2, st * PT : (st + 1) * PT],
                            qT[:, 2 * jp : 2 * jp + 2, lb * NB : (lb + 1) * NB],
                            start=(jp == 0),
                            stop=(jp == CCH // 2 - 1),
                            perf_mode=DR,
                        )
                    nc.scalar.activation(
                        out=a_pair[:, half, :],
                        in_=sps,
                        func=AF.Exp,
                        scale=SCALE / float(S * S),
                        bias=expb_t,
                    )

            def emit_zmm(lb, g):
                c = ctx[lb]
                nc.tensor.matmul(
                    c["zps"],
                    ones_col,
                    c["z4"].pop(g),
                    start=(g == 0),
                    stop=(g == NP // 2 - 1),
                )

            def emit_epilogue(lb):
                c = ctx[lb]
                # Z row -> [128, 4] columns, then cheap per-partition 1/Z
                zrow = tp.tile([1, NB], f32, tag="row", bufs=4, name=f"zrow{lb}")
                nc.vector.tensor_copy(zrow, c["zps"])
                zTr = tp.tile([PT, NB // PT], f32, tag="zTr", bufs=2)
                row_to_col(zrow, zTr, f"zT{lb}")
                zT = tp.tile([PT, NB // PT], f32, tag="zT", bufs=2)
                nc.vector.reciprocal(zT, zTr)
                # O~ accumulators -> fp8 pairs (scaled 1/64) for DR out-proj
                obf8 = []
                for mp in range(CCH // 2):
                    ot = tp.tile([PT, 2, NB], f8, tag="obf", bufs=2, name=f"obf{mp}")
                    nc.vector.tensor_scalar_mul(
                        ot[:, 0, :], c["ops"][2 * mp], 1.0 / 64.0
                    )
                    nc.vector.tensor_scalar_mul(
                        ot[:, 1, :], c["ops"][2 * mp + 1], 1.0 / 64.0
                    )
                    obf8.append(ot)
                # final projection; normalize by 1/Z and add bias+residual
                for sub in range(NB // PT):
                    t = lb * (NB // PT) + sub
                    yps = psp.tile(
                        [PT, NB], f32, tag=("py" if sub % 2 == 0 else "pz"),
                        bufs=1, name=f"yps{sub % 2}",
                    )
                    for mp in range(CCH // 2):
                        nc.tensor.matmul(
                            yps,
                            obf8[mp][:, :, sub * PT : (sub + 1) * PT],
                            wb["o"][:, 2 * mp : 2 * mp + 2, :],
                            start=(mp == 0),
                            stop=(mp == CCH // 2 - 1),
                            perf_mode=DR,
                        )
                    xrb = tp.tile([PT, C], f32, tag="xrb", bufs=2)
                    nc.vector.tensor_add(xrb, c["xr4"][:, sub, :], bo_bc)
                    yt = tp.tile([PT, C], f32, tag="yt", bufs=2)
                    nc.vector.scalar_tensor_tensor(
                        out=yt,
                        in0=yps,
                        scalar=zT[:, sub : sub + 1],
                        in1=xrb,
                        op0=mybir.AluOpType.mult,
                        op1=mybir.AluOpType.add,
                    )
                    nc.sync.dma_start(out=y_d[t * PT : (t + 1) * PT, :], in_=yt)
                del ctx[lb]

            emit_scores(0, 0)
            for gi in range(NLB * NP):
                lb, p = divmod(gi, NP)
                c = ctx[lb]
                if gi + 1 < NLB * NP:
                    emit_scores((gi + 1) // NP, (gi + 1) % NP)
                # Z matmul for pair-group (p-3)//2 — deferred so the DVE
                # pair-sums are long done and never stall the PE queue
                if p >= 3 and p % 2 == 1:
                    emit_zmm(lb, (p - 3) // 2)
                a_pair = c["a"].pop(p)
                for m in range(CCH):
                    nc.tensor.matmul(
                        c["ops"][m],
                        v_sb[:, 2 * p : 2 * p + 2, m * PT : (m + 1) * PT],
                        a_pair,
                        start=(p == 0),
                        stop=(p == NP - 1),
                        perf_mode=DR,
                    )
                # DVE pair-sum for Z
                zp = tp.tile([PT, NB], bf16, tag="zp", bufs=3, name=f"zp{p % 3}")
                nc.vector.tensor_add(zp, a_pair[:, 0, :], a_pair[:, 1, :])
                c["zp"][p] = zp
                if p % 2 == 1:
                    g = p // 2
                    z4 = tp.tile([PT, NB], bf16, tag="z4", bufs=3)
                    nc.vector.tensor_add(z4, c["zp"].pop(p - 1), c["zp"].pop(p))
                    c["z4"][g] = z4
                if p == NP - 1:
                    emit_zmm(lb, NP // 2 - 1)
                    emit_epilogue(lb)

    nc.compile()
    return nc


_NC_CACHE = None


def _get_program():
    global _NC_CACHE
    if _NC_CACHE is None:
        _NC_CACHE = build_program()
    return _NC_CACHE


def make_in_maps(inputs):
    import ml_dtypes

    hs = np.ascontiguousarray(np.asarray(inputs["hidden_states"], np.float32))
    ws = {n: np.ascontiguousarray(np.asarray(inputs["w" + n], np.float32)) for n in "qkvo"}
    bs = {n: np.ascontiguousarray(np.asarray(inputs["b" + n], np.float32)) for n in "qkvo"}
    gsc = np.ascontiguousarray(np.asarray(inputs["gn_scale"], np.float32))
    gbi = np.ascontiguousarray(np.asarray(inputs["gn_bias"], np.float32))
    eye = np.eye(PT, dtype=ml_dtypes.float8_e4m3)
    eg = np.zeros((G, C), np.float32)
    eg[np.arange(C) // GS, np.arange(C)] = 1.0
    in_maps = []
    for core in range(NCORES):
        b, h = core // 2, core % 2
        xb = hs[b].reshape(L, C)
        x_roll = np.ascontiguousarray(
            np.roll(xb, -h * LQ, axis=0).astype(ml_dtypes.bfloat16)
        )
        m = {"x": x_roll, "gn_scale": gsc, "gn_bias": gbi, "egrp": eg, "eye": eye}
        for n in "qkvo":
            m["w" + n] = ws[n]
            m["b" + n] = bs[n]
        in_maps.append(m)
    return in_maps


def assemble(results):
    out = np.empty((B, L, C), np.float32)
    for core in range(NCORES):
        b, h = core // 2, core % 2
        out[b, h * LQ : (h + 1) * LQ] = results[core]["y"]
    return out.reshape(B, HH, WW, C)


def kernel(**inputs):
    from concourse.bass_utils import run_bass_kernel_spmd

    nc = _get_program()
    in_maps = make_in_maps(inputs)
    res = run_bass_kernel_spmd(nc, in_maps, list(range(NCORES)))
    return assemble(res.results)


if __name__ == "__main__":
    rng = np.random.default_rng(0)
    s = 1.0 / np.sqrt(C)
    inputs = {
        "hidden_states": rng.standard_normal((B, HH, WW, C), np.float32),
        "gn_scale": np.ones(C, np.float32),
        "gn_bias": np.zeros(C, np.float32),
    }
    for n in "qkvo":
        inputs["w" + n] = (rng.standard_normal((C, C)) * s).astype(np.float32)
        inputs["b" + n] = np.zeros(C, np.float32)
    out = kernel(**inputs)
    print(out.shape, out.dtype)
